# revision 1
# baseline (speedup 1.0000x reference)
"""DiffiT transformer block kernel for 8 Trainium2 NeuronCores.

Data-parallel over the B=64 window axis (8 windows per core). Activations
are feature-major ([channel, token]) so every linear contracts over the
SBUF partition axis. Q/K stay feature-major with heads packed at a 96-row
stride (so each head's 72 rows sit at 32-aligned partition bases and the
per-head score matmuls can slice them legally); V is produced token-major
into per-head slots with an appended ones-column, so O^T = V_aug.T @ P^T
yields the softmax denominator as row 72. Per-token scalars (LN mean/rstd,
softmax 1/l) are broadcast across partitions with K=1 ones-matmuls on the
PE. Dense matmuls run bf16; the residual stream stays fp32; small fixup
matmuls use float32r (full-rate fp32 at free-dim >= 256).

All biases and the time-token conditioning (c @ qkvt^T + biases) enter as
rank-1 (K=1) matmul fixups folded into the PSUM accumulations.
"""

import math
from contextlib import ExitStack

import numpy as np
import ml_dtypes

import concourse.bass as bass
import concourse.mybir as mybir
import concourse.tile as tile
from concourse import bacc
from concourse import bass_utils

F32 = mybir.dt.float32
F32R = mybir.dt.float32r
BF16 = mybir.dt.bfloat16
NPBF16 = ml_dtypes.bfloat16
AF = mybir.ActivationFunctionType

P = 128
WS = 16
N = 256            # tokens per window
C = 1152           # hidden
H = 16             # heads
DH = 72            # head dim
HS = 96            # head stride in the QK packing (32-aligned, >= DH)
MLP = 4608
EPS = 1e-6
B = 64
NCORES = 8
NW = B // NCORES   # windows per core
KC = C // P        # 9  k-tiles over the hidden dim
QKM = 2 * H * HS // P   # 24 m-tiles over packed Q+K (96-stride)
KOFF = QKM // 2    # first K-side m-tile
M1T = MLP // P     # 36 fc1 row tiles
SCALE = 1.0 / math.sqrt(DH)


def _r(ap):
    """view a 4-byte fp32 AP as float32r for full-rate PE matmuls"""
    return ap.bitcast(F32R)


def _qk_pieces(h):
    """32-aligned partition pieces covering head h's 72 rows in the
    96-stride packing: [(subtile, base, length), ...]; piece legality:
    base 0 any len, base 64 len<=64, base 32/96 len<=32."""
    start, end = HS * h, HS * h + DH
    out = []
    while start < end:
        sub, base = divmod(start, P)
        ln = min(end - start, P - base)
        if base == 64:
            ln = min(ln, 64)
        elif base in (32, 96):
            ln = min(ln, 32)
        elif base != 0:
            raise AssertionError(base)
        out.append((sub, base, ln))
        start += ln
    return out


def build_program(nw=NW, sim_gelu=False):
    nc = bacc.Bacc("TRN2", target_bir_lowering=False, debug=False,
                   num_devices=NCORES)

    # register the layernorm epsilon as a const AP (activation float biases
    # other than 0.0/1.0 need one), same pattern as Bass.__init__
    eps_t = nc.alloc_sbuf_tensor("const-eps", [P, 1], F32)
    nc.gpsimd.memset(eps_t.ap(), EPS)
    nc.const_aps.aps[(F32, EPS)] = eps_t.ap()
    nc.all_engine_barrier()

    def din(name, shape, dt):
        return nc.dram_tensor(name, shape, dt, kind="ExternalInput").ap()

    xT = din("xT", [nw, P, KC, N], F32)          # x, feature-major
    xTb = din("xTb", [nw, P, KC, N], BF16)       # x, bf16 copy for LN stats
    cT = din("cT", [10, P, nw], BF16)            # c augmented with ones row
    wct = din("wct", [10, P, 4224], BF16)        # qkvt^T reordered + bias row
    wqk = din("wqk", [QKM, P, KC, P], BF16)      # qkv^T QK part, 96-stride
    wv = din("wv", [4, P, KC, 288], BF16)        # qkv^T V part, chunk-major
    expb = din("expb", [H, P, 2, N], BF16)       # exp(rel-pos bias)^T per head
    wps = din("wps", [KC, P, H, P], BF16)        # proj^T, head-slot padded
    w1c = din("w1c", [M1T, P, KC, P], BF16)      # fc1^T pre-chunked
    w2 = din("w2", [KC, P, M1T, P], BF16)        # fc2^T, pm-chunked
    f1b = din("f1b", [P, M1T], F32)              # fc1 bias, per-partition
    b2 = din("b2", [1, 2 * C], BF16)             # proj_b ++ fc2_b
    outT = nc.dram_tensor("outT", [nw, P, KC, N], F32,
                          kind="ExternalOutput").ap()

    NPAIR = nw // 2
    W2N = 2 * N        # tokens per window pair

    with tile.TileContext(nc) as tc, ExitStack() as ctx:
        keep = ctx.enter_context(tc.tile_pool(name="keep", bufs=1))
        dram = ctx.enter_context(tc.tile_pool(name="dram", bufs=1,
                                              space="DRAM"))

        ones_b = keep.tile([1, W2N], BF16, tag="ones_b")  # bf16 rhs of K=1
        ones_c = keep.tile([P, 1], BF16, tag="ones_c")    # lhsT of column sums
        nc.gpsimd.memset(ones_b[:], 1.0)
        nc.gpsimd.memset(ones_c[:], 1.0)
        bias2 = keep.tile([1, 2 * C], BF16, tag="bias2")
        nc.sync.dma_start(bias2[:], b2[:])
        f1bs = keep.tile([P, M1T], F32, tag="f1bs")
        nc.sync.dma_start(f1bs[:], f1b[:])

        tdram = dram.tile([nw, 4224], BF16)
        xpd = dram.tile([nw, P, KC, N], F32)     # x after attention branch
        xpdb = dram.tile([nw, P, KC, N], BF16)   # bf16 shadow for LN2

        # ---- phase 0: conditioning T = c_aug @ W_ct ----------------------
        with tc.tile_pool(name="ph0", bufs=2) as p0, \
             tc.tile_pool(name="ph0p", bufs=2, space="PSUM") as pp0:
            caug = p0.tile([P, 10, nw], BF16, tag="caug")
            nc.sync.dma_start(caug[:], cT.rearrange("k p w -> p k w"))
            tsb = p0.tile([8, 4224], BF16, tag="tsb")
            for i in range(9):
                n0, nl = i * 512, min(512, 4224 - i * 512)
                tps = pp0.tile([8, 512], F32, tag="tps")
                for k in range(10):
                    wt = p0.tile([P, 512], BF16, tag="wctt")
                    nc.sync.dma_start(wt[:, :nl], wct[k, :, n0:n0 + nl])
                    nc.tensor.matmul(tps[:nw, :nl], caug[:, k, :], wt[:, :nl],
                                     start=(k == 0), stop=(k == 9))
                nc.scalar.activation(tsb[:nw, n0:n0 + nl], tps[:nw, :nl],
                                     AF.Copy)
            nc.sync.dma_start(tdram[:, :], tsb[:nw, :])

        # ---- layernorm for a window pair -> PSUM broadcast [P, W2N] ------
        # acc-tile layout: [:, :N]+[:, N:] hold the two windows; returns one
        # [P, W2N] psum tile pair (rstd bcast, -mean*rstd bcast)
        def ln_pair(pool, rows, accp, fetch, tag):
            """fetch(s) -> [P, W2N] bf16 tile of the LN input, sub-tile s.
            Returns (rstd_bcast, -mean*rstd bcast) PSUM tiles."""
            ms0 = accp.tile([1, W2N], F32, tag="acc")
            ms1 = accp.tile([1, W2N], F32, tag="acc")
            for s in range(KC):
                xbs = fetch(s)
                xsq = pool.tile([P, W2N], BF16, tag=tag + "xsq")
                nc.vector.tensor_mul(xsq[:], xbs[:], xbs[:])
                nc.tensor.matmul(ms0[:], ones_c[:], xbs[:],
                                 start=(s == 0), stop=(s == KC - 1))
                nc.tensor.matmul(ms1[:], ones_c[:], xsq[:],
                                 start=(s == 0), stop=(s == KC - 1))
            mean = rows.tile([1, W2N], F32, tag="r_mean")
            ra = rows.tile([1, W2N], F32, tag="r_a")
            rb = rows.tile([1, W2N], F32, tag="r_b")
            nc.vector.tensor_scalar_mul(mean[:], ms0[:], 1.0 / C)
            nc.vector.tensor_scalar_mul(ra[:], ms1[:], 1.0 / C)   # E[x^2]
            nc.vector.tensor_mul(rb[:], mean[:], mean[:])         # mean^2
            nc.vector.tensor_sub(ra[:], ra[:], rb[:])             # var
            nc.scalar.activation(rb[:], ra[:], AF.Sqrt, bias=EPS) # sd
            nc.vector.reciprocal_approx_fast(ra[:], rb[:])        # 1/sd
            rstd = rows.tile([1, W2N], BF16, tag="r_rstd")
            nc.gpsimd.tensor_copy(rstd[:], ra[:])
            bneg = rows.tile([1, W2N], BF16, tag="r_bneg")
            nc.vector.scalar_tensor_tensor(
                bneg[:], mean[:], -1.0, rstd[:],
                mybir.AluOpType.mult, mybir.AluOpType.mult)
            bc = accp.tile([P, W2N], F32, tag="acc")
            nc.tensor.matmul(bc[:], ones_b[:1, :P], rstd[:],
                             start=True, stop=True)
            bb = accp.tile([P, W2N], F32, tag="acc")
            nc.tensor.matmul(bb[:], ones_b[:1, :P], bneg[:],
                             start=True, stop=True)
            return bc, bb

        # ==== attention superphase: per pair LN1 -> QKV -> attn -> proj ===
        with tc.tile_pool(name="sp", bufs=2) as sp, \
             tc.tile_pool(name="sp1", bufs=1) as sp1, \
             tc.tile_pool(name="spw", bufs=2) as spw, \
             tc.tile_pool(name="sps", bufs=3) as sps, \
             tc.tile_pool(name="spr", bufs=2) as spr, \
             tc.tile_pool(name="rows", bufs=1) as rows, \
             tc.tile_pool(name="accp", bufs=8, space="PSUM") as accp:
            def fetch_dram_bf16(src, w0, pool, tag):
                def fetch(s):
                    t = pool.tile([P, W2N], BF16, tag=tag)
                    for wh in range(2):
                        nc.sync.dma_start(t[:, wh * N:(wh + 1) * N],
                                          src[w0 + wh, :, s, :])
                    return t
                return fetch

            for pr in range(NPAIR):
                w0 = 2 * pr
                f_x = fetch_dram_bf16(xTb, w0, spw, "xbs")
                bc, bb = ln_pair(spw, rows, accp, f_x, "ln1")
                hw = sp.tile([P, KC, W2N], BF16, tag="hw")
                for s in range(KC):
                    xbs = f_x(s)
                    nc.vector.tensor_mul(hw[:, s, :], xbs[:], bc[:])
                    nc.vector.tensor_add(hw[:, s, :], hw[:, s, :], bb[:])
                # QK (96-stride packed), N = both windows
                qkst = sp1.tile([P, QKM, W2N], BF16, tag="qkst")
                for m in range(QKM):
                    wt = spw.tile([P, KC, P], BF16, tag="wqkt")
                    nc.sync.dma_start(wt[:], wqk[m])
                    t1m = spw.tile([1, 2, P], BF16, tag="t1m")
                    nc.sync.dma_start(
                        t1m[:], tdram[w0:w0 + 2, P * m:P * (m + 1)]
                        .unsqueeze(0))
                    qs = accp.tile([P, W2N], F32, tag="acc")
                    for k in range(KC):
                        nc.tensor.matmul(qs[:], wt[:, k, :], hw[:, k, :],
                                         start=(k == 0), stop=False)
                    nc.tensor.matmul(qs[:, :N], t1m[:1, 0, :],
                                     ones_b[:1, :N], start=False, stop=False)
                    nc.tensor.matmul(qs[:, N:], t1m[:1, 1, :],
                                     ones_b[:1, :N], start=False, stop=True)
                    nc.scalar.activation(qkst[:, m, :], qs[:], AF.Copy)
                # V token-major into per-head slots (ones in col 0)
                vsl = sp1.tile([P, 2, 2, H, 73], BF16, tag="vsl")
                nc.vector.memset(vsl[:, :, :, :, 0:1], 1.0)
                for nch in range(4):
                    wvt = spw.tile([P, KC, 288], BF16, tag="wvt")
                    nc.sync.dma_start(wvt[:], wv[nch])
                    t1vc = spw.tile([1, 2, 288], BF16, tag="t1vc")
                    nc.sync.dma_start(
                        t1vc[:],
                        tdram[w0:w0 + 2, 3072 + 288 * nch:3072 + 288 * (nch + 1)]
                        .unsqueeze(0))
                    for tch in range(4):       # token chunks of the pair
                        wh, ms = divmod(tch, 2)
                        vs = accp.tile([P, W2N], F32, tag="acc")
                        tsl = slice(tch * P, (tch + 1) * P)
                        for k in range(KC):
                            nc.tensor.matmul(vs[:, :288], hw[:, k, tsl],
                                             wvt[:, k, :],
                                             start=(k == 0), stop=False)
                        nc.tensor.matmul(
                            vs[:, :288], ones_b[:1, :P], t1vc[:1, wh, :],
                            start=False, stop=True)
                        nc.scalar.activation(
                            vsl[:, wh, ms, 4 * nch:4 * nch + 4, 1:73],
                            vs[:, :288].rearrange("p (h d) -> p h d", d=72),
                            AF.Copy)
                # attention, head-outer so expb loads once per pair
                ost = sp1.tile([P, H, W2N], BF16, tag="ost")
                nc.gpsimd.memset(ost[64:, :, :], 0.0)
                for h in range(H):
                    ebt = sps.tile([P, 2, N], BF16, tag="ebt")
                    nc.sync.dma_start(ebt[:], expb[h])
                    pieces = _qk_pieces(h)
                    for wh in range(2):
                        nsl = slice(wh * N, (wh + 1) * N)
                        pt = sps.tile([P, 2, N], BF16, tag="pt")
                        po = accp.tile([P, W2N], F32, tag="acc")
                        for ms in range(2):
                            ssp = accp.tile([P, W2N], F32, tag="acc")
                            msl = slice(wh * N + ms * P, wh * N + (ms + 1) * P)
                            for i, (sub, base, ln) in enumerate(pieces):
                                nc.tensor.matmul(
                                    ssp[:, :N],
                                    qkst[base:base + ln, KOFF + sub, msl],
                                    qkst[base:base + ln, sub, nsl],
                                    start=(i == 0),
                                    stop=(i == len(pieces) - 1),
                                    tile_position=(base, 0))
                            nc.scalar.activation(pt[:, ms, :], ssp[:, :N],
                                                 AF.Exp, scale=SCALE)
                            nc.vector.tensor_mul(pt[:, ms, :], pt[:, ms, :],
                                                 ebt[:, ms, :])
                        for ms in range(2):
                            nc.tensor.matmul(po[:73, :N],
                                             vsl[:, wh, ms, h, :],
                                             pt[:, ms, :],
                                             start=(ms == 0), stop=(ms == 1))
                        linv = spr.tile([1, N], F32, tag="linv")
                        nc.vector.reciprocal_approx_fast(linv[:], po[0:1, :N])
                        pbs = spr.tile([P, N], F32, tag="pbs")
                        nc.gpsimd.partition_broadcast(pbs[:73, :], linv[:],
                                                      channels=73)
                        nc.scalar.activation(ost[:73, h, nsl], po[:73, :N],
                                             AF.Copy)
                        nc.vector.tensor_mul(ost[:73, h, nsl],
                                             ost[:73, h, nsl], pbs[:73, :])
                # proj + residual -> xpd (fp32) + xpdb (bf16 shadow)
                for pc in range(KC):
                    wpt = spw.tile([P, H, P], BF16, tag="wpt")
                    nc.sync.dma_start(wpt[:], wps[pc])
                    yps = accp.tile([P, W2N], F32, tag="acc")
                    for h in range(H):
                        nc.tensor.matmul(yps[:], wpt[:, h, :], ost[:, h, :],
                                         start=(h == 0), stop=False)
                    nc.tensor.matmul(yps[:], bias2[:1, P * pc:P * (pc + 1)],
                                     ones_b[:1, :W2N], start=False, stop=True)
                    xres = spw.tile([P, 2, N], F32, tag="xres")
                    for wh in range(2):
                        nc.sync.dma_start(xres[:, wh, :],
                                          xT[w0 + wh, :, pc, :])
                    nc.vector.tensor_add(
                        xres[:], xres[:],
                        yps[:].rearrange("p (u n) -> p u n", n=N))
                    xrb = spw.tile([P, 2, N], BF16, tag="xrb")
                    nc.scalar.activation(
                        xrb[:].rearrange("p u n -> p (u n)"),
                        xres[:].rearrange("p u n -> p (u n)"), AF.Copy)
                    for wh in range(2):
                        nc.sync.dma_start(xpd[w0 + wh, :, pc, :],
                                          xres[:, wh, :])
                        nc.sync.dma_start(xpdb[w0 + wh, :, pc, :],
                                          xrb[:, wh, :])
                # LN2 from the bf16 shadow
                f_xp = fetch_dram_bf16(xpdb, w0, spw, "xpbs")
                bc2, bb2 = ln_pair(spw, rows, accp, f_xp, "ln2")
                hp = sp.tile([P, KC, W2N], BF16, tag="hw")
                for s in range(KC):
                    xbs = f_xp(s)
                    nc.vector.tensor_mul(hp[:, s, :], xbs[:], bc2[:])
                    nc.vector.tensor_add(hp[:, s, :], hp[:, s, :], bb2[:])
                # fc1 -> gelu -> h2a
                h2a = sp1.tile([P, M1T, W2N], BF16, tag="h2a")
                for m1 in range(M1T):
                    w1t = spw.tile([P, KC, P], BF16, tag="w1t")
                    nc.sync.dma_start(w1t[:], w1c[m1])
                    ps1 = accp.tile([P, W2N], F32, tag="acc")
                    for k in range(KC):
                        nc.tensor.matmul(ps1[:], w1t[:, k, :], hp[:, k, :],
                                         start=(k == 0), stop=(k == KC - 1))
                    h2c = h2a[:, m1, :]
                    if not sim_gelu:
                        nc.scalar.activation(h2c, ps1[:], AF.Gelu_apprx_tanh,
                                             bias=f1bs[:, m1:m1 + 1])
                    else:
                        u = rows.tile([P, W2N], F32, tag="gelu_u")
                        nc.vector.tensor_add(
                            u[:], ps1[:],
                            f1bs[:, m1:m1 + 1].to_broadcast((P, W2N)))
                        t3 = rows.tile([P, W2N], F32, tag="gelu_t3")
                        nc.vector.tensor_mul(t3[:], u[:], u[:])
                        nc.vector.tensor_mul(t3[:], t3[:], u[:])
                        nc.vector.scalar_tensor_tensor(
                            t3[:], t3[:], 0.044715, u[:],
                            mybir.AluOpType.mult, mybir.AluOpType.add)
                        nc.scalar.activation(t3[:], t3[:], AF.Tanh,
                                             scale=0.7978845608028654)
                        nc.vector.scalar_tensor_tensor(
                            t3[:], t3[:], 1.0, u[:],
                            mybir.AluOpType.add, mybir.AluOpType.mult)
                        nc.vector.tensor_scalar_mul(h2c, t3[:], 0.5)
                # fc2 + residual + output
                for pm in range(KC):
                    w2t = spw.tile([P, M1T, P], BF16, tag="w2t")
                    nc.sync.dma_start(w2t[:], w2[pm])
                    ps2 = accp.tile([P, W2N], F32, tag="acc")
                    for m1 in range(M1T):
                        nc.tensor.matmul(ps2[:], w2t[:, m1, :], h2a[:, m1, :],
                                         start=(m1 == 0), stop=False)
                    nc.tensor.matmul(
                        ps2[:], bias2[:1, C + P * pm:C + P * (pm + 1)],
                        ones_b[:1, :W2N], start=False, stop=True)
                    xps = spw.tile([P, 2, N], F32, tag="xps")
                    for wh in range(2):
                        nc.sync.dma_start(xps[:, wh, :],
                                          xpd[w0 + wh, :, pm, :])
                    ot = spw.tile([P, 2, N], F32, tag="ot")
                    nc.vector.tensor_add(
                        ot[:], xps[:],
                        ps2[:].rearrange("p (u n) -> p u n", n=N))
                    for wh in range(2):
                        nc.sync.dma_start(outT[w0 + wh, :, pm, :],
                                          ot[:, wh, :])

    nc.compile()
    return nc


# ---------------------------------------------------------------------------
# host side
# ---------------------------------------------------------------------------

def _qk_colmap():
    m = np.full(2 * H * HS, -1, np.int64)
    for h in range(H):
        m[HS * h:HS * h + DH] = np.arange(72 * h, 72 * h + 72)
        m[H * HS + HS * h:H * HS + HS * h + DH] = \
            np.arange(C + 72 * h, C + 72 * h + 72)
    return m


def _prep_core_inputs(x_c, c_c, wdict):
    """x_c: [nw, N, C], c_c: [nw, C] -> per-core input map"""
    nw = x_c.shape[0]
    xT = np.ascontiguousarray(
        x_c.transpose(0, 2, 1).reshape(nw, KC, P, N).transpose(
            0, 2, 1, 3)).astype(np.float32)
    caug = np.zeros((nw, 1280), np.float32)
    caug[:, :C] = c_c
    caug[:, C] = 1.0
    cT = np.ascontiguousarray(caug.T.reshape(10, P, nw)).astype(NPBF16)
    return {"xT": xT, "xTb": xT.astype(NPBF16), "cT": cT, **wdict}


def _prep_weights(qkv_w, qkv_b, qkvt_w, qkvt_b, rpb_table, rel_idx,
                  proj_w, proj_b, fc1_w, fc1_b, fc2_w, fc2_b):
    qkmap = _qk_colmap()
    amap = np.concatenate([qkmap, np.arange(2 * C, 3 * C)])  # 4224 cols
    valid = amap >= 0

    wct = np.zeros((1280, 4224), np.float32)
    wct[:C, valid] = qkvt_w[amap[valid], :].T
    wct[C, valid] = (qkv_b + qkvt_b)[amap[valid]]
    wct = wct.reshape(10, P, 4224).astype(NPBF16)

    nqk = 2 * H * HS
    wqkT = np.zeros((C, nqk), np.float32)
    wqkT[:, valid[:nqk]] = qkv_w[qkmap[valid[:nqk]], :].T
    wqk = np.ascontiguousarray(
        wqkT.reshape(KC, P, QKM, P).transpose(2, 1, 0, 3)).astype(NPBF16)

    wv = np.ascontiguousarray(
        qkv_w[2 * C:, :].T.reshape(KC, P, 4, 288).transpose(
            2, 1, 0, 3)).astype(NPBF16)

    bias = rpb_table[rel_idx]                      # [N(n), N(m), H]
    expb = np.ascontiguousarray(
        np.exp(bias).transpose(2, 1, 0).reshape(H, 2, P, N).transpose(
            0, 2, 1, 3)).astype(NPBF16)

    wp_sl = np.zeros((P, H, C), np.float32)        # [slot-row d, head, p]
    for h in range(H):
        wp_sl[1:73, h, :] = proj_w[:, 72 * h:72 * h + 72].T
    wps = np.ascontiguousarray(
        wp_sl.reshape(P, H, KC, P).transpose(2, 0, 1, 3)).astype(NPBF16)

    w1c = np.ascontiguousarray(
        fc1_w.T.reshape(KC, P, M1T, P).transpose(2, 1, 0, 3)).astype(NPBF16)
    w2 = np.ascontiguousarray(
        fc2_w.T.reshape(M1T, P, KC, P).transpose(2, 1, 0, 3)).astype(NPBF16)
    f1b = np.ascontiguousarray(fc1_b.reshape(M1T, P).T).astype(np.float32)
    b2 = np.concatenate([proj_b, fc2_b]).reshape(1, 2 * C).astype(NPBF16)

    return {"wct": wct, "wqk": wqk, "wv": wv, "expb": expb, "wps": wps,
            "w1c": w1c, "w2": w2, "f1b": f1b, "b2": b2}


_PROGRAM = None


def kernel(x, c, qkv_w, qkv_b, qkvt_w, qkvt_b, rpb_table, proj_w, proj_b,
           fc1_w, fc1_b, fc2_w, fc2_b, rel_idx, _trace=False):
    global _PROGRAM
    x = np.asarray(x, np.float32)
    c = np.asarray(c, np.float32)
    wdict = _prep_weights(
        np.asarray(qkv_w, np.float32), np.asarray(qkv_b, np.float32),
        np.asarray(qkvt_w, np.float32), np.asarray(qkvt_b, np.float32),
        np.asarray(rpb_table, np.float32), np.asarray(rel_idx),
        np.asarray(proj_w, np.float32), np.asarray(proj_b, np.float32),
        np.asarray(fc1_w, np.float32), np.asarray(fc1_b, np.float32),
        np.asarray(fc2_w, np.float32), np.asarray(fc2_b, np.float32))

    if _PROGRAM is None:
        _PROGRAM = build_program(NW)
    nc = _PROGRAM

    in_maps = []
    for core in range(NCORES):
        sl = slice(core * NW, (core + 1) * NW)
        in_maps.append(_prep_core_inputs(x[sl], c[sl], wdict))

    res = bass_utils.run_bass_kernel_spmd(
        nc, in_maps, core_ids=list(range(NCORES)), trace=_trace)

    out = np.empty((B, N, C), np.float32)
    for core in range(NCORES):
        oT = res.results[core]["outT"]            # [NW, P, KC, N]
        out[core * NW:(core + 1) * NW] = \
            oT.transpose(0, 2, 1, 3).reshape(NW, C, N).transpose(0, 2, 1)
    if _trace:
        return out, res
    return out



# revision 19
# speedup vs baseline: 1.0851x; 1.0851x over previous
"""DiffiT transformer block kernel for 8 Trainium2 NeuronCores.

Data-parallel over the B=64 window axis (8 windows per core). Weight-
resident superphases per core:

  A) conditioning: TT = (c_aug @ W_ct)^T feature-major tiles for the QK
     bias (DVE per-partition broadcast at QKV evacuation), and vbs =
     V-part bias in head-slot rows (folded in AFTER softmax
     normalization -- exact, since sum_m P[n,m] = 1).
  B) attention, window-granular (free dim 256): LN1 -> QKV (96-stride
     packed Q/K, token-major V slots with ones col 72) -> per-head
     scores/softmax/PV -> PE permutation-repack of head-slot O into
     dense feature-major O -> dense proj + residual. All attention
     weights resident in SBUF; exp(rel-pos-bias) streamed per head.
  C) MLP, pair-granular (free dim 512): LN2 -> fc1+gelu -> fc2 +
     residual. fp8e4m3 DoubleRow (2x PE) with weight scale 32 folded
     out at PSUM evacuation; weights resident.

Activations are feature-major ([channel, token]) so every linear
contracts over the SBUF partition axis. LN stats come from bf16 shadows
of the fp32 residual stream via ones-matmuls.
"""

import math
from contextlib import ExitStack

import numpy as np
import ml_dtypes

import concourse.bass as bass
import concourse.mybir as mybir
import concourse.tile as tile
from concourse import bacc
from concourse import bass_utils

F32 = mybir.dt.float32
BF16 = mybir.dt.bfloat16
FP8 = mybir.dt.float8e4
NPBF16 = ml_dtypes.bfloat16
NPFP8 = ml_dtypes.float8_e4m3
AF = mybir.ActivationFunctionType
DR = mybir.MatmulPerfMode.DoubleRow

P = 128
WS = 16
N = 256            # tokens per window
C = 1152           # hidden
H = 16             # heads
DH = 72            # head dim
HS = 96            # head stride in the QK packing (32-aligned, >= DH)
MLP = 4608
EPS = 1e-6
B = 64
NCORES = 8
NW = B // NCORES   # windows per core
KC = C // P        # 9 k-tiles over the hidden dim
QKM = 2 * H * HS // P   # 24 m-tiles over packed Q+K (96-stride)
KOFF = QKM // 2    # first K-side m-tile
M1T = MLP // P     # 36 fc1 row tiles
SCALE = 1.0 / math.sqrt(DH)
W2N = 2 * N

# fp8 MLP config
FP8_FC1 = True
FC1_COMP = True    # 2-pass activation-compensated fp8 fc1 (hp_hi + hp_lo)
FP8_FC2 = False
SW1 = 32.0         # fc1 weight scale
SW2 = 32.0         # fc2 weight scale
KP1 = 5            # fc1 doublerow k-pair tiles (9 k-tiles padded to 10)
KP2 = 18           # fc2 doublerow k-pair tiles (36 k-tiles)


def _qk_pieces(h):
    """32-aligned partition pieces covering head h's 72 rows in the
    96-stride packing: [(subtile, base, length), ...]"""
    start, end = HS * h, HS * h + DH
    out = []
    while start < end:
        sub, base = divmod(start, P)
        ln = min(end - start, P - base)
        if base == 64:
            ln = min(ln, 64)
        elif base in (32, 96):
            ln = min(ln, 32)
        elif base != 0:
            raise AssertionError(base)
        out.append((sub, base, ln))
        start += ln
    return out


def _repack_pieces():
    """(pc, h, col0, r0, ln): dense tile pc cols [col0, col0+ln) take ost
    slot-h rows [r0, r0+ln)  (slot row r = d, dense channel 72h + d)."""
    out = []
    for h in range(H):
        c0, c1 = DH * h, DH * h + DH
        while c0 < c1:
            pc, col0 = divmod(c0, P)
            ln = min(c1 - c0, P - col0)
            out.append((pc, h, col0, 1 + (c0 - DH * h), ln))
            c0 += ln
    return out


def build_program(nw=NW):
    nc = bacc.Bacc("TRN2", target_bir_lowering=False, debug=False,
                   num_devices=NCORES)

    # register the layernorm epsilon as a const AP
    eps_t = nc.alloc_sbuf_tensor("const-eps", [P, 1], F32)
    nc.gpsimd.memset(eps_t.ap(), EPS)
    nc.const_aps.aps[(F32, EPS)] = eps_t.ap()
    nc.all_engine_barrier()

    def din(name, shape, dt):
        return nc.dram_tensor(name, shape, dt, kind="ExternalInput").ap()

    xT = din("xT", [nw, P, KC, N], F32)          # x, feature-major
    xTb = din("xTb", [nw, P, KC, N], BF16)       # bf16 shadow for LN1
    cT = din("cT", [10, P, nw], BF16)            # c augmented with ones row
    wct = din("wct", [10, P, 3072], BF16)        # qkvt^T QK part + bias row
    wcv = din("wcv", [10, P, H, 73], BF16)       # qkvt^T V part, slot rows
    wqk = din("wqk", [P, QKM * KC * P], BF16)    # QK weights, 96-stride flat
    wv = din("wv", [P, 4 * KC * 288], BF16)      # V weights, chunk flat
    expb = din("expb", [H, P, 2, N], BF16)       # exp(rel-pos bias)^T per head
    wpd = din("wpd", [P, KC * KC * P], BF16)     # dense proj^T flat
    perm = din("perm", [P, 24 * P], BF16)        # ost-slot -> dense repack
    f1b = din("f1b", [P, M1T], F32)              # fc1 bias, per-partition
    b2T = din("b2T", [P, 2 * KC], F32)           # proj_b ++ fc2_b tile cols
    if FP8_FC1:
        w1 = din("w1", [P, M1T * KP1 * 2 * P], FP8)
    else:
        w1 = din("w1", [P, M1T * KC * P], BF16)
    if FP8_FC2:
        w2 = din("w2", [P, KC * KP2 * 2 * P], FP8)
    else:
        w2 = din("w2", [KC, P, M1T, P], BF16)
    outT = nc.dram_tensor("outT", [nw, P, KC, N], F32,
                          kind="ExternalOutput").ap()

    rp_pieces = _repack_pieces()
    assert len(rp_pieces) == 24

    with tile.TileContext(nc) as tc, ExitStack() as ctx:
        keep = ctx.enter_context(tc.tile_pool(name="keep", bufs=1))
        dram = ctx.enter_context(tc.tile_pool(name="dram", bufs=1,
                                              space="DRAM"))

        ones_b = keep.tile([1, W2N], BF16, tag="ones_b")
        ones_c = keep.tile([P, 1], BF16, tag="ones_c")
        nc.gpsimd.memset(ones_b[:], 1.0)
        nc.gpsimd.memset(ones_c[:], 1.0)
        f1bs = keep.tile([P, M1T], F32, tag="f1bs")
        nc.sync.dma_start(f1bs[:], f1b[:])
        b2s = keep.tile([P, 2 * KC], F32, tag="b2s")
        nc.sync.dma_start(b2s[:], b2T[:])
        TT = keep.tile([P, QKM, nw], F32, tag="TT")     # QK bias, feat-major
        vbs = keep.tile([P, H, nw], F32, tag="vbs")     # V bias, slot rows

        xpd = dram.tile([nw, P, KC, N], F32)     # x after attention branch
        xpdb = dram.tile([nw, P, KC, N], BF16)   # bf16 shadow for LN2

        # ================= phase B scope (incl. conditioning) =========
        with tc.tile_pool(name="att", bufs=1) as att, \
             tc.tile_pool(name="aw", bufs=2) as aw, \
             tc.tile_pool(name="aw3", bufs=3) as aw3, \
             tc.tile_pool(name="rows", bufs=2) as rows, \
             tc.tile_pool(name="stp", bufs=2, space="PSUM") as stp, \
             tc.tile_pool(name="accp", bufs=5, space="PSUM") as accp:

            def fetch_x(w):
                xw = aw.tile([P, KC, N], F32, tag="xw")
                xbw = aw.tile([P, KC, N], BF16, tag="xbw")
                for s in range(KC):
                    nc.sync.dma_start(xw[:, s, :], xT[w, :, s, :])
                    nc.sync.dma_start(xbw[:, s, :], xTb[w, :, s, :])
                return xw, xbw

            def ln_stats(xbw):
                """-> (bc, bb) PSUM broadcast tiles [P, :N]"""
                ms0 = stp.tile([P, 512], F32, tag="st")
                ms1 = stp.tile([P, 512], F32, tag="st")
                for s in range(KC):
                    xsq = aw.tile([P, N], BF16, tag="xsq")
                    nc.vector.tensor_mul(xsq[:], xbw[:, s, :], xbw[:, s, :])
                    nc.tensor.matmul(ms0[:1, :N], ones_c[:], xbw[:, s, :],
                                     start=(s == 0), stop=(s == KC - 1))
                    nc.tensor.matmul(ms1[:1, :N], ones_c[:], xsq[:],
                                     start=(s == 0), stop=(s == KC - 1))
                mean = rows.tile([1, N], F32, tag="r_mean")
                ra = rows.tile([1, N], F32, tag="r_a")
                rb = rows.tile([1, N], F32, tag="r_b")
                nc.vector.tensor_scalar_mul(mean[:], ms0[:1, :N], 1.0 / C)
                nc.vector.tensor_scalar_mul(ra[:], ms1[:1, :N], 1.0 / C)
                nc.vector.tensor_mul(rb[:], mean[:], mean[:])
                nc.vector.tensor_sub(ra[:], ra[:], rb[:])
                nc.scalar.activation(rb[:], ra[:], AF.Sqrt, bias=EPS)
                nc.vector.reciprocal_approx_fast(ra[:], rb[:])
                rstd = rows.tile([1, N], BF16, tag="r_rstd")
                nc.gpsimd.tensor_copy(rstd[:], ra[:])
                bneg = rows.tile([1, N], BF16, tag="r_bneg")
                nc.vector.scalar_tensor_tensor(
                    bneg[:], mean[:], -1.0, rstd[:],
                    mybir.AluOpType.mult, mybir.AluOpType.mult)
                bc = stp.tile([P, 512], F32, tag="st")
                nc.tensor.matmul(bc[:, :N], ones_b[:1, :P], rstd[:],
                                 start=True, stop=True)
                bb = stp.tile([P, 512], F32, tag="st")
                nc.tensor.matmul(bb[:, :N], ones_b[:1, :P], bneg[:],
                                 start=True, stop=True)
                return bc, bb

            def ln_apply(xbw, bc, bb):
                hw = aw.tile([P, KC, N], BF16, tag="hw")
                for s in range(KC):
                    nc.vector.tensor_mul(hw[:, s, :], xbw[:, s, :], bc[:, :N])
                    nc.vector.tensor_add(hw[:, s, :], hw[:, s, :], bb[:, :N])
                return hw

            # window-0 activations first in the DMA queue
            xw, xbw = fetch_x(0)

            # ---- conditioning: TT (QK bias) + vbs (V bias) -----------
            caug = keep.tile([P, 10, nw], BF16, tag="caug")
            nc.sync.dma_start(caug[:], cT.rearrange("k p w -> p k w"))
            for j in range(QKM):
                tps = accp.tile([P, 512], F32, tag="acc")
                for k in range(10):
                    wt = aw3.tile([P, P], BF16, tag="wctt")
                    nc.sync.dma_start(wt[:], wct[k, :, j * P:(j + 1) * P])
                    nc.tensor.matmul(tps[:, :nw], wt[:], caug[:, k, :],
                                     start=(k == 0), stop=(k == 9))
                nc.scalar.activation(TT[:, j, :], tps[:, :nw], AF.Copy)
            for h in range(H):
                vps = accp.tile([P, 512], F32, tag="acc")
                for k in range(10):
                    wt = aw3.tile([P, 73], BF16, tag="wctv")
                    nc.sync.dma_start(wt[:], wcv[k, :, h, :])
                    nc.tensor.matmul(vps[:73, :nw], wt[:], caug[:, k, :],
                                     start=(k == 0), stop=(k == 9))
                nc.scalar.activation(vbs[:73, h, :], vps[:73, :nw], AF.Copy)

            # ---- resident attention weights (chunked) ----------------
            wqk_sb = att.tile([P, QKM * KC * P], BF16, tag="wqk")
            for m in range(QKM):
                s = slice(m * KC * P, (m + 1) * KC * P)
                nc.sync.dma_start(wqk_sb[:, s], wqk[:, s])
            wv_sb = att.tile([P, 4 * KC * 288], BF16, tag="wv")
            for i in range(4):
                s = slice(i * KC * 288, (i + 1) * KC * 288)
                nc.sync.dma_start(wv_sb[:, s], wv[:, s])
            wpd_sb = att.tile([P, KC * KC * P], BF16, tag="wpd")
            for i in range(KC):
                s = slice(i * KC * P, (i + 1) * KC * P)
                nc.sync.dma_start(wpd_sb[:, s], wpd[:, s])
            perm_sb = att.tile([P, 24 * P], BF16, tag="perm")
            nc.sync.dma_start(perm_sb[:], perm[:])

            qkst = att.tile([P, QKM, N], BF16, tag="qkst")
            vsl = att.tile([P, 2, H, 73], BF16, tag="vsl")
            ost = att.tile([P, H, N], BF16, tag="ost")
            od = att.tile([P, KC, N], BF16, tag="od")
            nc.vector.memset(vsl[:, :, :, 0:1], 1.0)     # ones col (denom)

            pc_pieces = {}
            for idx, (pc, h, col0, r0, ln) in enumerate(rp_pieces):
                pc_pieces.setdefault(pc, []).append((idx, h, col0, r0, ln))

            bc, bb = ln_stats(xbw)
            hw = ln_apply(xbw, bc, bb)
            for w in range(nw):
                # QKV (Q/K 96-stride packed), bias via DVE broadcast of TT
                for m in range(QKM):
                    qs = accp.tile([P, 512], F32, tag="acc")
                    for k in range(KC):
                        nc.tensor.matmul(
                            qs[:, :N], wqk_sb[:, (m * KC + k) * P:
                                              (m * KC + k + 1) * P],
                            hw[:, k, :], start=(k == 0), stop=(k == KC - 1))
                    nc.vector.tensor_add(
                        qkst[:, m, :], qs[:, :N],
                        TT[:, m, w:w + 1].to_broadcast((P, N)))
                # V token-major into per-head slots (data rows 0..71)
                for nch in range(4):
                    for tch in range(2):
                        vs = accp.tile([P, 512], F32, tag="acc")
                        tsl = slice(tch * P, (tch + 1) * P)
                        for k in range(KC):
                            nc.tensor.matmul(
                                vs[:, :288], hw[:, k, tsl],
                                wv_sb[:, (nch * KC + k) * 288:
                                      (nch * KC + k + 1) * 288],
                                start=(k == 0), stop=(k == KC - 1))
                        nc.scalar.activation(
                            vsl[:, tch, 4 * nch:4 * nch + 4, 1:73],
                            vs[:, :288].rearrange("p (h d) -> p h d", d=72),
                            AF.Copy)
                # attention per head
                for h in range(H):
                    ebt = aw3.tile([P, 2, N], BF16, tag="ebt")
                    nc.sync.dma_start(ebt[:], expb[h])
                    pieces = _qk_pieces(h)
                    pt = aw.tile([P, 2, N], BF16, tag="pt")
                    po = accp.tile([P, 512], F32, tag="acc")
                    for ms in range(2):
                        ssp = accp.tile([P, 512], F32, tag="acc")
                        msl = slice(ms * P, (ms + 1) * P)
                        for i, (sub, base, ln) in enumerate(pieces):
                            nc.tensor.matmul(
                                ssp[:, :N],
                                qkst[base:base + ln, KOFF + sub, msl],
                                qkst[base:base + ln, sub, :],
                                start=(i == 0), stop=(i == len(pieces) - 1),
                                tile_position=(base, 0))
                        nc.scalar.activation(pt[:, ms, :], ssp[:, :N],
                                             AF.Exp, scale=SCALE)
                        nc.vector.tensor_mul(pt[:, ms, :], pt[:, ms, :],
                                             ebt[:, ms, :])
                    for ms in range(2):
                        nc.tensor.matmul(po[:73, :N], vsl[:, ms, h, :],
                                         pt[:, ms, :],
                                         start=(ms == 0), stop=(ms == 1))
                    linv = rows.tile([1, N], F32, tag="linv")
                    nc.vector.reciprocal_approx_fast(linv[:], po[0:1, :N])
                    pbs = aw.tile([P, N], F32, tag="pbs")
                    nc.gpsimd.partition_broadcast(pbs[:73, :], linv[:],
                                                  channels=73)
                    nc.vector.tensor_mul(ost[:73, h, :], po[:73, :N],
                                         pbs[:73, :])
                    nc.vector.tensor_add(
                        ost[:73, h, :], ost[:73, h, :],
                        vbs[:73, h, w:w + 1].to_broadcast((73, N)))
                # hoisted LN1 for w+1 (overlaps repack/proj below)
                if w + 1 < nw:
                    xw2, xbw2 = fetch_x(w + 1)
                    bc, bb = ln_stats(xbw2)
                    hw = ln_apply(xbw2, bc, bb)
                # repack head-slot O into dense feature-major od
                for pc in range(KC):
                    dn = accp.tile([P, 512], F32, tag="acc")
                    pcs = pc_pieces[pc]
                    for i, (idx, h, col0, r0, ln) in enumerate(pcs):
                        nc.tensor.matmul(
                            dn[:, :N], perm_sb[:73, idx * P:(idx + 1) * P],
                            ost[:73, h, :],
                            start=(i == 0), stop=(i == len(pcs) - 1))
                    nc.scalar.activation(od[:, pc, :], dn[:, :N], AF.Copy)
                # dense proj + residual -> xpd / xpdb
                for pc in range(KC):
                    yps = accp.tile([P, 512], F32, tag="acc")
                    for k in range(KC):
                        nc.tensor.matmul(
                            yps[:, :N], wpd_sb[:, (pc * KC + k) * P:
                                               (pc * KC + k + 1) * P],
                            od[:, k, :], start=(k == 0), stop=(k == KC - 1))
                    nc.vector.tensor_add(xw[:, pc, :], xw[:, pc, :],
                                         yps[:, :N])
                    xpo = aw3.tile([P, N], F32, tag="xpo")
                    nc.scalar.activation(xpo[:], xw[:, pc, :], AF.Identity,
                                         bias=b2s[:, pc:pc + 1])
                    xpob = aw3.tile([P, N], BF16, tag="xpob")
                    nc.scalar.activation(xpob[:], xw[:, pc, :], AF.Identity,
                                         bias=b2s[:, pc:pc + 1])
                    nc.sync.dma_start(xpd[w, :, pc, :], xpo[:])
                    nc.sync.dma_start(xpdb[w, :, pc, :], xpob[:])
                if w + 1 < nw:
                    xw = xw2

        # ================= phase C: MLP (pair-granular) ===============
        NPAIR = nw // 2
        KH = 10 if FP8_FC1 else KC
        HDT = FP8 if FP8_FC1 else BF16
        H2DT = FP8 if FP8_FC2 else BF16
        with tc.tile_pool(name="mlp", bufs=1) as mp, \
             tc.tile_pool(name="mw", bufs=2) as mw, \
             tc.tile_pool(name="mw3", bufs=3) as mw3, \
             tc.tile_pool(name="mrows", bufs=2) as mrows, \
             tc.tile_pool(name="mstp", bufs=2, space="PSUM") as mstp, \
             tc.tile_pool(name="maccp", bufs=6, space="PSUM") as maccp:

            if FP8_FC1:
                w1_sb = mp.tile([P, M1T * KP1 * 2 * P], FP8, tag="w1")
                for m in range(M1T):
                    s = slice(m * KP1 * 2 * P, (m + 1) * KP1 * 2 * P)
                    nc.sync.dma_start(w1_sb[:, s], w1[:, s])
            else:
                w1_sb = mp.tile([P, M1T * KC * P], BF16, tag="w1")
                for m in range(M1T):
                    s = slice(m * KC * P, (m + 1) * KC * P)
                    nc.sync.dma_start(w1_sb[:, s], w1[:, s])
            if FP8_FC2:
                w2_sb = mp.tile([P, KC * KP2 * 2 * P], FP8, tag="w2")
                for m in range(KC):
                    s = slice(m * KP2 * 2 * P, (m + 1) * KP2 * 2 * P)
                    nc.sync.dma_start(w2_sb[:, s], w2[:, s])

            h2a = mp.tile([P, M1T, W2N], H2DT, tag="h2a")

            def fetch_xq(pr):
                xq = mw.tile([P, KC, 2, N], F32, tag="xq")
                xqb = mw.tile([P, KC, 2, N], BF16, tag="xqb")
                for s in range(KC):
                    for u in range(2):
                        nc.sync.dma_start(xq[:, s, u, :],
                                          xpd[2 * pr + u, :, s, :])
                        nc.sync.dma_start(xqb[:, s, u, :],
                                          xpdb[2 * pr + u, :, s, :])
                return xq, xqb

            def ln2(xqb):
                xqbf = xqb.rearrange("p s u n -> p s (u n)")
                ms0 = mstp.tile([P, 512], F32, tag="st")
                ms1 = mstp.tile([P, 512], F32, tag="st")
                for s in range(KC):
                    xsq = mw.tile([P, W2N], BF16, tag="mxsq")
                    nc.vector.tensor_mul(xsq[:], xqbf[:, s, :], xqbf[:, s, :])
                    nc.tensor.matmul(ms0[:1, :], ones_c[:], xqbf[:, s, :],
                                     start=(s == 0), stop=(s == KC - 1))
                    nc.tensor.matmul(ms1[:1, :], ones_c[:], xsq[:],
                                     start=(s == 0), stop=(s == KC - 1))
                mean = mrows.tile([1, W2N], F32, tag="m_mean")
                ra = mrows.tile([1, W2N], F32, tag="m_a")
                rb = mrows.tile([1, W2N], F32, tag="m_b")
                nc.vector.tensor_scalar_mul(mean[:], ms0[:1, :], 1.0 / C)
                nc.vector.tensor_scalar_mul(ra[:], ms1[:1, :], 1.0 / C)
                nc.vector.tensor_mul(rb[:], mean[:], mean[:])
                nc.vector.tensor_sub(ra[:], ra[:], rb[:])
                nc.scalar.activation(rb[:], ra[:], AF.Sqrt, bias=EPS)
                nc.vector.reciprocal_approx_fast(ra[:], rb[:])
                rstd = mrows.tile([1, W2N], BF16, tag="m_rstd")
                nc.gpsimd.tensor_copy(rstd[:], ra[:])
                bneg = mrows.tile([1, W2N], BF16, tag="m_bneg")
                nc.vector.scalar_tensor_tensor(
                    bneg[:], mean[:], -1.0, rstd[:],
                    mybir.AluOpType.mult, mybir.AluOpType.mult)
                bc = mstp.tile([P, 512], F32, tag="st")
                nc.tensor.matmul(bc[:], ones_b[:1, :P], rstd[:],
                                 start=True, stop=True)
                bb = mstp.tile([P, 512], F32, tag="st")
                nc.tensor.matmul(bb[:], ones_b[:1, :P], bneg[:],
                                 start=True, stop=True)
                hp = mw.tile([P, KH, W2N], HDT, tag="hp")
                hp_lo = None
                if FP8_FC1 and FC1_COMP:
                    hp_lo = mw.tile([P, KH, W2N], FP8, tag="hplo")
                    nc.vector.memset(hp_lo[:, KC:, :], 0.0)
                if KH > KC:
                    nc.vector.memset(hp[:, KC:, :], 0.0)
                for s in range(KC):
                    if HDT == BF16:
                        nc.vector.tensor_mul(hp[:, s, :], xqbf[:, s, :], bc[:])
                        nc.vector.tensor_add(hp[:, s, :], hp[:, s, :], bb[:])
                    else:
                        th = mw.tile([P, W2N], BF16, tag="mth")
                        nc.vector.tensor_mul(th[:], xqbf[:, s, :], bc[:])
                        nc.vector.tensor_add(th[:], th[:], bb[:])
                        nc.vector.tensor_copy(hp[:, s, :], th[:])
                        if FC1_COMP:
                            terr = mw.tile([P, W2N], BF16, tag="mterr")
                            nc.vector.tensor_sub(terr[:], th[:], hp[:, s, :])
                            nc.vector.tensor_copy(hp_lo[:, s, :], terr[:])
                return hp, hp_lo

            xq, xqb = fetch_xq(0)
            hp, hp_lo = ln2(xqb)
            for pr in range(NPAIR):
                # fc1 -> gelu -> h2a
                for m1 in range(M1T):
                    ps1 = maccp.tile([P, 512], F32, tag="acc")
                    if FP8_FC1:
                        hps = [hp, hp_lo] if FC1_COMP else [hp]
                        for pi, hh in enumerate(hps):
                            for i in range(KP1):
                                lw = w1_sb[:, (m1 * KP1 + i) * 2 * P:
                                           (m1 * KP1 + i + 1) * 2 * P] \
                                    .rearrange("p (j c) -> p j c", j=2)
                                nc.tensor.matmul(
                                    ps1[:], lw, hh[:, 2 * i:2 * i + 2, :],
                                    start=(pi == 0 and i == 0),
                                    stop=(pi == len(hps) - 1 and i == KP1 - 1),
                                    perf_mode=DR)
                        gsc = 1.0 / SW1
                    else:
                        for k in range(KC):
                            nc.tensor.matmul(
                                ps1[:], w1_sb[:, (m1 * KC + k) * P:
                                              (m1 * KC + k + 1) * P],
                                hp[:, k, :], start=(k == 0),
                                stop=(k == KC - 1))
                        gsc = 1.0
                    nc.scalar.activation(h2a[:, m1, :], ps1[:],
                                         AF.Gelu_apprx_tanh,
                                         bias=f1bs[:, m1:m1 + 1], scale=gsc)
                # hoisted LN2 for pr+1 (overlaps fc2)
                if pr + 1 < NPAIR:
                    xq2, xqb2 = fetch_xq(pr + 1)
                    hp, hp_lo = ln2(xqb2)
                # fc2 + residual + output
                for pm in range(KC):
                    ps2 = maccp.tile([P, 512], F32, tag="acc")
                    if FP8_FC2:
                        for i in range(KP2):
                            lw = w2_sb[:, (pm * KP2 + i) * 2 * P:
                                       (pm * KP2 + i + 1) * 2 * P] \
                                .rearrange("p (j c) -> p j c", j=2)
                            nc.tensor.matmul(ps2[:], lw,
                                             h2a[:, 2 * i:2 * i + 2, :],
                                             start=(i == 0),
                                             stop=(i == KP2 - 1),
                                             perf_mode=DR)
                        osc = 1.0 / SW2
                    else:
                        w2t = mw.tile([P, M1T, P], BF16, tag="w2t")
                        nc.sync.dma_start(w2t[:], w2[pm])
                        for m1 in range(M1T):
                            nc.tensor.matmul(ps2[:], w2t[:, m1, :],
                                             h2a[:, m1, :],
                                             start=(m1 == 0),
                                             stop=(m1 == M1T - 1))
                        osc = 1.0
                    tb = mw3.tile([P, W2N], F32, tag="tb")
                    nc.scalar.activation(tb[:], ps2[:], AF.Identity,
                                         bias=b2s[:, KC + pm:KC + pm + 1],
                                         scale=osc)
                    ot = mw3.tile([P, 2, N], F32, tag="ot")
                    nc.vector.tensor_add(
                        ot[:], xq[:, pm, :, :],
                        tb[:].rearrange("p (u n) -> p u n", n=N))
                    for u in range(2):
                        nc.sync.dma_start(outT[2 * pr + u, :, pm, :],
                                          ot[:, u, :])
                if pr + 1 < NPAIR:
                    xq = xq2

    nc.compile()
    return nc


# ---------------------------------------------------------------------------
# host side
# ---------------------------------------------------------------------------

def _qk_colmap():
    m = np.full(2 * H * HS, -1, np.int64)
    for h in range(H):
        m[HS * h:HS * h + DH] = np.arange(DH * h, DH * h + DH)
        m[H * HS + HS * h:H * HS + HS * h + DH] = \
            np.arange(C + DH * h, C + DH * h + DH)
    return m


def _prep_core_inputs(x_c, c_c, wdict):
    """x_c: [nw, N, C], c_c: [nw, C] -> per-core input map"""
    nw = x_c.shape[0]
    xT = np.ascontiguousarray(
        x_c.transpose(0, 2, 1).reshape(nw, KC, P, N).transpose(
            0, 2, 1, 3)).astype(np.float32)
    caug = np.zeros((nw, 1280), np.float32)
    caug[:, :C] = c_c
    caug[:, C] = 1.0
    cT = np.ascontiguousarray(caug.T.reshape(10, P, nw)).astype(NPBF16)
    return {"xT": xT, "xTb": xT.astype(NPBF16), "cT": cT, **wdict}


def _prep_weights(qkv_w, qkv_b, qkvt_w, qkvt_b, rpb_table, rel_idx,
                  proj_w, proj_b, fc1_w, fc1_b, fc2_w, fc2_b):
    qkmap = _qk_colmap()
    valid = qkmap >= 0

    wct = np.zeros((1280, 3072), np.float32)
    wct[:C, valid] = qkvt_w[qkmap[valid], :].T
    wct[C, valid] = (qkv_b + qkvt_b)[qkmap[valid]]
    wct = wct.reshape(10, P, 3072).astype(NPBF16)

    wcv = np.zeros((1280, H, 73), np.float32)
    vmap = np.arange(2 * C, 3 * C).reshape(H, DH)
    for h in range(H):
        wcv[:C, h, 1:73] = qkvt_w[vmap[h], :].T
        wcv[C, h, 1:73] = (qkv_b + qkvt_b)[vmap[h]]
    wcv = wcv.reshape(10, P, H, 73).astype(NPBF16)

    nqk = 2 * H * HS
    wqkT = np.zeros((C, nqk), np.float32)
    wqkT[:, valid[:nqk]] = qkv_w[qkmap[valid[:nqk]], :].T
    # flat [P, QKM*KC*P]: tile (m, k)[p, j] = wqkT[k*P+p, m*P+j]
    wqk = np.ascontiguousarray(
        wqkT.reshape(KC, P, QKM, P).transpose(1, 2, 0, 3).reshape(
            P, QKM * KC * P)).astype(NPBF16)

    # flat [P, 4*KC*288]: tile (nch, k)[p, j] = wvT[k*P+p, nch*288+j]
    wvT = qkv_w[2 * C:, :].T
    wv = np.ascontiguousarray(
        wvT.reshape(KC, P, 4, 288).transpose(1, 2, 0, 3).reshape(
            P, 4 * KC * 288)).astype(NPBF16)

    bias = rpb_table[rel_idx]                      # [N(n), N(m), H]
    expb = np.ascontiguousarray(
        np.exp(bias).transpose(2, 1, 0).reshape(H, 2, P, N).transpose(
            0, 2, 1, 3)).astype(NPBF16)

    # dense proj flat [P, KC*KC*P]: tile (pc, k)[p, j] = proj_w[pc*P+j, k*P+p]
    wpd = np.zeros((P, KC * KC * P), np.float32)
    pw = proj_w.reshape(KC, P, KC, P)              # [pc, j, k, p]
    for pc in range(KC):
        for k in range(KC):
            wpd[:, (pc * KC + k) * P:(pc * KC + k + 1) * P] = pw[pc, :, k, :].T
    wpd = wpd.astype(NPBF16)

    permf = np.zeros((P, 24 * P), np.float32)
    for idx, (pc, h, col0, r0, ln) in enumerate(_repack_pieces()):
        for d in range(ln):
            permf[r0 + d, idx * P + col0 + d] = 1.0
    perm = permf.astype(NPBF16)

    if FP8_FC1:
        w1s = fc1_w * SW1                          # [MLP, C]
        w1p = np.zeros((M1T, KP1 * 2, P, P), np.float32)  # [m, kk, p, j]
        for m in range(M1T):
            for kk in range(KC):
                w1p[m, kk] = w1s[m * P:(m + 1) * P, kk * P:(kk + 1) * P].T
        w1 = np.ascontiguousarray(
            w1p.transpose(2, 0, 1, 3).reshape(P, M1T * KP1 * 2 * P)) \
            .astype(NPFP8)
    else:
        w1 = np.zeros((P, M1T * KC * P), np.float32)
        f1 = fc1_w.reshape(M1T, P, KC, P)          # [m, j, k, p]
        for m in range(M1T):
            for k in range(KC):
                w1[:, (m * KC + k) * P:(m * KC + k + 1) * P] = f1[m, :, k, :].T
        w1 = w1.astype(NPBF16)

    if FP8_FC2:
        w2s = fc2_w * SW2                          # [C, MLP]
        w2p = np.zeros((KC, KP2 * 2, P, P), np.float32)
        for pm in range(KC):
            for kk in range(M1T):
                w2p[pm, kk] = w2s[pm * P:(pm + 1) * P, kk * P:(kk + 1) * P].T
        w2 = np.ascontiguousarray(
            w2p.transpose(2, 0, 1, 3).reshape(P, KC * KP2 * 2 * P)) \
            .astype(NPFP8)
    else:
        w2 = np.ascontiguousarray(
            fc2_w.T.reshape(M1T, P, KC, P).transpose(2, 1, 0, 3)) \
            .astype(NPBF16)

    f1bv = np.ascontiguousarray(fc1_b.reshape(M1T, P).T).astype(np.float32)
    b2T = np.ascontiguousarray(
        np.concatenate([proj_b.reshape(KC, P), fc2_b.reshape(KC, P)])
        .T).astype(np.float32)

    return {"wct": wct, "wcv": wcv, "wqk": wqk, "wv": wv, "expb": expb,
            "wpd": wpd, "perm": perm, "w1": w1, "w2": w2, "f1b": f1bv,
            "b2T": b2T}


_PROGRAM = None


def kernel(x, c, qkv_w, qkv_b, qkvt_w, qkvt_b, rpb_table, proj_w, proj_b,
           fc1_w, fc1_b, fc2_w, fc2_b, rel_idx, _trace=False):
    global _PROGRAM
    x = np.asarray(x, np.float32)
    c = np.asarray(c, np.float32)
    wdict = _prep_weights(
        np.asarray(qkv_w, np.float32), np.asarray(qkv_b, np.float32),
        np.asarray(qkvt_w, np.float32), np.asarray(qkvt_b, np.float32),
        np.asarray(rpb_table, np.float32), np.asarray(rel_idx),
        np.asarray(proj_w, np.float32), np.asarray(proj_b, np.float32),
        np.asarray(fc1_w, np.float32), np.asarray(fc1_b, np.float32),
        np.asarray(fc2_w, np.float32), np.asarray(fc2_b, np.float32))

    if _PROGRAM is None:
        _PROGRAM = build_program(NW)
    nc = _PROGRAM

    in_maps = []
    for core in range(NCORES):
        sl = slice(core * NW, (core + 1) * NW)
        in_maps.append(_prep_core_inputs(x[sl], c[sl], wdict))

    res = bass_utils.run_bass_kernel_spmd(
        nc, in_maps, core_ids=list(range(NCORES)), trace=_trace)

    out = np.empty((B, N, C), np.float32)
    for core in range(NCORES):
        oT = res.results[core]["outT"]            # [NW, P, KC, N]
        out[core * NW:(core + 1) * NW] = \
            oT.transpose(0, 2, 1, 3).reshape(NW, C, N).transpose(0, 2, 1)
    if _trace:
        return out, res
    return out


# revision 23
# speedup vs baseline: 1.1223x; 1.0343x over previous
"""DiffiT transformer block kernel for 8 Trainium2 NeuronCores.

Data-parallel over the B=64 window axis (8 windows per core). Weight-
resident superphases per core:

  A) conditioning: TT = (c_aug @ W_ct)^T feature-major tiles for the QK
     bias (DVE per-partition broadcast at QKV evacuation), and vbs =
     V-part bias in head-slot rows (folded in AFTER softmax
     normalization -- exact, since sum_m P[n,m] = 1).
  B) attention, window-granular (free dim 256): LN1 -> QKV (96-stride
     packed Q/K, token-major V slots with ones col 72) -> per-head
     scores/softmax/PV -> PE permutation-repack of head-slot O into
     dense feature-major O -> dense proj + residual. All attention
     weights resident in SBUF; exp(rel-pos-bias) streamed per head.
  C) MLP, pair-granular (free dim 512): LN2 -> fc1+gelu -> fc2 +
     residual. fp8e4m3 DoubleRow (2x PE) with weight scale 32 folded
     out at PSUM evacuation; weights resident.

Activations are feature-major ([channel, token]) so every linear
contracts over the SBUF partition axis. LN stats come from bf16 shadows
of the fp32 residual stream via ones-matmuls.
"""

import math
from contextlib import ExitStack

import numpy as np
import ml_dtypes

import concourse.bass as bass
import concourse.mybir as mybir
import concourse.tile as tile
from concourse import bacc
from concourse import bass_utils

F32 = mybir.dt.float32
BF16 = mybir.dt.bfloat16
FP8 = mybir.dt.float8e4
NPBF16 = ml_dtypes.bfloat16
NPFP8 = ml_dtypes.float8_e4m3
AF = mybir.ActivationFunctionType
DR = mybir.MatmulPerfMode.DoubleRow

P = 128
WS = 16
N = 256            # tokens per window
C = 1152           # hidden
H = 16             # heads
DH = 72            # head dim
HS = 96            # head stride in the QK packing (32-aligned, >= DH)
MLP = 4608
EPS = 1e-6
B = 64
NCORES = 8
NW = B // NCORES   # windows per core
KC = C // P        # 9 k-tiles over the hidden dim
QKM = 2 * H * HS // P   # 24 m-tiles over packed Q+K (96-stride)
KOFF = QKM // 2    # first K-side m-tile
M1T = MLP // P     # 36 fc1 row tiles
SCALE = 1.0 / math.sqrt(DH)
W2N = 2 * N

# fp8 MLP config
FP8_FC1 = True
FC1_COMP = True    # 2-pass activation-compensated fp8 fc1 (hp_hi + hp_lo)
FP8_FC2 = False
SW1 = 32.0         # fc1 weight scale
SW2 = 32.0         # fc2 weight scale
KP1 = 5            # fc1 doublerow k-pair tiles (9 k-tiles padded to 10)
KP2 = 18           # fc2 doublerow k-pair tiles (36 k-tiles)


def _qk_pieces(h):
    """32-aligned partition pieces covering head h's 72 rows in the
    96-stride packing: [(subtile, base, length), ...]"""
    start, end = HS * h, HS * h + DH
    out = []
    while start < end:
        sub, base = divmod(start, P)
        ln = min(end - start, P - base)
        if base == 64:
            ln = min(ln, 64)
        elif base in (32, 96):
            ln = min(ln, 32)
        elif base != 0:
            raise AssertionError(base)
        out.append((sub, base, ln))
        start += ln
    return out


def _repack_pieces():
    """(pc, h, col0, r0, ln): dense tile pc cols [col0, col0+ln) take ost
    slot-h rows [r0, r0+ln)  (slot row r = d, dense channel 72h + d)."""
    out = []
    for h in range(H):
        c0, c1 = DH * h, DH * h + DH
        while c0 < c1:
            pc, col0 = divmod(c0, P)
            ln = min(c1 - c0, P - col0)
            out.append((pc, h, col0, 1 + (c0 - DH * h), ln))
            c0 += ln
    return out


def build_program(nw=NW):
    nc = bacc.Bacc("TRN2", target_bir_lowering=False, debug=False,
                   num_devices=NCORES)

    # register the layernorm epsilon as a const AP
    eps_t = nc.alloc_sbuf_tensor("const-eps", [P, 1], F32)
    nc.gpsimd.memset(eps_t.ap(), EPS)
    nc.const_aps.aps[(F32, EPS)] = eps_t.ap()
    nc.all_engine_barrier()

    def din(name, shape, dt):
        return nc.dram_tensor(name, shape, dt, kind="ExternalInput").ap()

    xT = din("xT", [nw, P, KC, N], F32)          # x, feature-major
    xTb = din("xTb", [nw, P, KC, N], BF16)       # bf16 shadow for LN1
    cT = din("cT", [10, P, nw], BF16)            # c augmented with ones row
    wct = din("wct", [10, P, 3072], BF16)        # qkvt^T QK part + bias row
    wcv = din("wcv", [10, P, H, 73], BF16)       # qkvt^T V part, slot rows
    wqk = din("wqk", [P, QKM * KC * P], BF16)    # QK weights, 96-stride flat
    wv = din("wv", [P, 4 * KC * 288], BF16)      # V weights, chunk flat
    expb = din("expb", [H, P, 2, N], BF16)       # exp(rel-pos bias)^T per head
    wpd = din("wpd", [P, KC * KC * P], BF16)     # dense proj^T flat
    perm = din("perm", [P, 24 * P], BF16)        # ost-slot -> dense repack
    f1b = din("f1b", [P, M1T], F32)              # fc1 bias, per-partition
    b2T = din("b2T", [P, 2 * KC], F32)           # proj_b ++ fc2_b tile cols
    if FP8_FC1:
        w1 = din("w1", [P, M1T * KP1 * 2 * P], FP8)
    else:
        w1 = din("w1", [P, M1T * KC * P], BF16)
    if FP8_FC2:
        w2 = din("w2", [P, KC * KP2 * 2 * P], FP8)
    else:
        w2 = din("w2", [KC, P, M1T, P], BF16)
    outT = nc.dram_tensor("outT", [nw, P, KC, N], F32,
                          kind="ExternalOutput").ap()

    rp_pieces = _repack_pieces()
    assert len(rp_pieces) == 24

    with tile.TileContext(nc) as tc, ExitStack() as ctx:
        keep = ctx.enter_context(tc.tile_pool(name="keep", bufs=1))
        dram = ctx.enter_context(tc.tile_pool(name="dram", bufs=1,
                                              space="DRAM"))

        ones_b = keep.tile([1, W2N], BF16, tag="ones_b")
        ones_c = keep.tile([P, 1], BF16, tag="ones_c")
        nc.gpsimd.memset(ones_b[:], 1.0)
        nc.gpsimd.memset(ones_c[:], 1.0)
        f1bs = keep.tile([P, M1T], F32, tag="f1bs")
        nc.sync.dma_start(f1bs[:], f1b[:])
        b2s = keep.tile([P, 2 * KC], F32, tag="b2s")
        nc.sync.dma_start(b2s[:], b2T[:])
        TT = keep.tile([P, QKM, nw], F32, tag="TT")     # QK bias, feat-major
        vbs = keep.tile([P, H, nw], F32, tag="vbs")     # V bias, slot rows

        xpd = dram.tile([nw, P, KC, N], F32)     # x after attention branch
        xpdb = dram.tile([nw, P, KC, N], BF16)   # bf16 shadow for LN2

        # ================= phase B scope (incl. conditioning) =========
        with tc.tile_pool(name="att", bufs=1) as att, \
             tc.tile_pool(name="aw", bufs=2) as aw, \
             tc.tile_pool(name="aw3", bufs=3) as aw3, \
             tc.tile_pool(name="rows", bufs=2) as rows, \
             tc.tile_pool(name="stp", bufs=2, space="PSUM") as stp, \
             tc.tile_pool(name="accp", bufs=5, space="PSUM") as accp:

            def fetch_x(w):
                xw = aw.tile([P, KC, N], F32, tag="xw")
                xbw = aw.tile([P, KC, N], BF16, tag="xbw")
                for s in range(KC):
                    nc.sync.dma_start(xw[:, s, :], xT[w, :, s, :])
                    nc.sync.dma_start(xbw[:, s, :], xTb[w, :, s, :])
                return xw, xbw

            def ln_stats(xbw):
                """-> (bc, bb) PSUM broadcast tiles [P, :N]"""
                ms0 = stp.tile([P, 512], F32, tag="st")
                ms1 = stp.tile([P, 512], F32, tag="st")
                for s in range(KC):
                    xsq = aw.tile([P, N], BF16, tag="xsq")
                    nc.vector.tensor_mul(xsq[:], xbw[:, s, :], xbw[:, s, :])
                    nc.tensor.matmul(ms0[:1, :N], ones_c[:], xbw[:, s, :],
                                     start=(s == 0), stop=(s == KC - 1))
                    nc.tensor.matmul(ms1[:1, :N], ones_c[:], xsq[:],
                                     start=(s == 0), stop=(s == KC - 1))
                mean = rows.tile([1, N], F32, tag="r_mean")
                ra = rows.tile([1, N], F32, tag="r_a")
                rb = rows.tile([1, N], F32, tag="r_b")
                nc.vector.tensor_scalar_mul(mean[:], ms0[:1, :N], 1.0 / C)
                nc.vector.tensor_scalar_mul(ra[:], ms1[:1, :N], 1.0 / C)
                nc.vector.tensor_mul(rb[:], mean[:], mean[:])
                nc.vector.tensor_sub(ra[:], ra[:], rb[:])
                nc.scalar.activation(rb[:], ra[:], AF.Sqrt, bias=EPS)
                nc.vector.reciprocal_approx_fast(ra[:], rb[:])
                rstd = rows.tile([1, N], BF16, tag="r_rstd")
                nc.gpsimd.tensor_copy(rstd[:], ra[:])
                bneg = rows.tile([1, N], BF16, tag="r_bneg")
                nc.vector.scalar_tensor_tensor(
                    bneg[:], mean[:], -1.0, rstd[:],
                    mybir.AluOpType.mult, mybir.AluOpType.mult)
                bc = stp.tile([P, 512], F32, tag="st")
                nc.tensor.matmul(bc[:, :N], ones_b[:1, :P], rstd[:],
                                 start=True, stop=True)
                bb = stp.tile([P, 512], F32, tag="st")
                nc.tensor.matmul(bb[:, :N], ones_b[:1, :P], bneg[:],
                                 start=True, stop=True)
                return bc, bb

            def ln_apply(xbw, bc, bb):
                hw = aw.tile([P, KC, N], BF16, tag="hw")
                for s in range(KC):
                    nc.vector.tensor_mul(hw[:, s, :], xbw[:, s, :], bc[:, :N])
                    nc.vector.tensor_add(hw[:, s, :], hw[:, s, :], bb[:, :N])
                return hw

            # window-0 activations first in the DMA queue
            xw, xbw = fetch_x(0)

            # ---- conditioning: TT (QK bias) + vbs (V bias) -----------
            caug = keep.tile([P, 10, nw], BF16, tag="caug")
            nc.sync.dma_start(caug[:], cT.rearrange("k p w -> p k w"))
            for j in range(QKM):
                tps = accp.tile([P, 512], F32, tag="acc")
                for k in range(10):
                    wt = aw3.tile([P, P], BF16, tag="wctt")
                    nc.sync.dma_start(wt[:], wct[k, :, j * P:(j + 1) * P])
                    nc.tensor.matmul(tps[:, :nw], wt[:], caug[:, k, :],
                                     start=(k == 0), stop=(k == 9))
                nc.scalar.activation(TT[:, j, :], tps[:, :nw], AF.Copy)
            for h in range(H):
                vps = accp.tile([P, 512], F32, tag="acc")
                for k in range(10):
                    wt = aw3.tile([P, 73], BF16, tag="wctv")
                    nc.sync.dma_start(wt[:], wcv[k, :, h, :])
                    nc.tensor.matmul(vps[:73, :nw], wt[:], caug[:, k, :],
                                     start=(k == 0), stop=(k == 9))
                nc.scalar.activation(vbs[:73, h, :], vps[:73, :nw], AF.Copy)

            # ---- resident attention weights (chunked) ----------------
            wqk_sb = att.tile([P, QKM * KC * P], BF16, tag="wqk")
            for m in range(QKM):
                s = slice(m * KC * P, (m + 1) * KC * P)
                nc.sync.dma_start(wqk_sb[:, s], wqk[:, s])
            wv_sb = att.tile([P, 4 * KC * 288], BF16, tag="wv")
            for i in range(4):
                s = slice(i * KC * 288, (i + 1) * KC * 288)
                nc.sync.dma_start(wv_sb[:, s], wv[:, s])
            wpd_sb = att.tile([P, KC * KC * P], BF16, tag="wpd")
            for i in range(KC):
                s = slice(i * KC * P, (i + 1) * KC * P)
                nc.sync.dma_start(wpd_sb[:, s], wpd[:, s])
            perm_sb = att.tile([P, 24 * P], BF16, tag="perm")
            nc.sync.dma_start(perm_sb[:], perm[:])

            qkst = att.tile([P, QKM, N], BF16, tag="qkst")
            vsl = att.tile([P, 2, H, 73], BF16, tag="vsl")
            ost = att.tile([P, H, N], BF16, tag="ost")
            od = att.tile([P, KC, N], BF16, tag="od")
            pts = att.tile([P, H, 2, N], BF16, tag="pts")
            nc.vector.memset(vsl[:, :, :, 0:1], 1.0)     # ones col (denom)

            pc_pieces = {}
            for idx, (pc, h, col0, r0, ln) in enumerate(rp_pieces):
                pc_pieces.setdefault(pc, []).append((idx, h, col0, r0, ln))

            bc, bb = ln_stats(xbw)
            hw = ln_apply(xbw, bc, bb)
            for w in range(nw):
                # QKV (Q/K 96-stride packed), bias via DVE broadcast of TT
                for m in range(QKM):
                    qs = accp.tile([P, 512], F32, tag="acc")
                    for k in range(KC):
                        nc.tensor.matmul(
                            qs[:, :N], wqk_sb[:, (m * KC + k) * P:
                                              (m * KC + k + 1) * P],
                            hw[:, k, :], start=(k == 0), stop=(k == KC - 1))
                    nc.vector.tensor_add(
                        qkst[:, m, :], qs[:, :N],
                        TT[:, m, w:w + 1].to_broadcast((P, N)))
                # V token-major into per-head slots (data rows 0..71)
                for nch in range(4):
                    for tch in range(2):
                        vs = accp.tile([P, 512], F32, tag="acc")
                        tsl = slice(tch * P, (tch + 1) * P)
                        for k in range(KC):
                            nc.tensor.matmul(
                                vs[:, :288], hw[:, k, tsl],
                                wv_sb[:, (nch * KC + k) * 288:
                                      (nch * KC + k + 1) * 288],
                                start=(k == 0), stop=(k == KC - 1))
                        nc.scalar.activation(
                            vsl[:, tch, 4 * nch:4 * nch + 4, 1:73],
                            vs[:, :288].rearrange("p (h d) -> p h d", d=72),
                            AF.Copy)
                # scores + softmax for all heads (PE never waits on softmax)
                for h in range(H):
                    ebt = aw3.tile([P, 2, N], BF16, tag="ebt")
                    nc.sync.dma_start(ebt[:], expb[h])
                    pieces = _qk_pieces(h)
                    for ms in range(2):
                        ssp = accp.tile([P, 512], F32, tag="acc")
                        msl = slice(ms * P, (ms + 1) * P)
                        for i, (sub, base, ln) in enumerate(pieces):
                            nc.tensor.matmul(
                                ssp[:, :N],
                                qkst[base:base + ln, KOFF + sub, msl],
                                qkst[base:base + ln, sub, :],
                                start=(i == 0), stop=(i == len(pieces) - 1),
                                tile_position=(base, 0))
                        nc.scalar.activation(pts[:, h, ms, :], ssp[:, :N],
                                             AF.Exp, scale=SCALE)
                        nc.vector.tensor_mul(pts[:, h, ms, :],
                                             pts[:, h, ms, :], ebt[:, ms, :])
                # hoisted LN1 for w+1 (PE filler between phases)
                if w + 1 < nw:
                    xw2, xbw2 = fetch_x(w + 1)
                    bc, bb = ln_stats(xbw2)
                    hw = ln_apply(xbw2, bc, bb)
                # PV for all heads (pt inputs long ready)
                for h in range(H):
                    po = accp.tile([P, 512], F32, tag="acc")
                    for ms in range(2):
                        nc.tensor.matmul(po[:73, :N], vsl[:, ms, h, :],
                                         pts[:, h, ms, :],
                                         start=(ms == 0), stop=(ms == 1))
                    lrow = rows.tile([1, N], F32, tag="lrow")
                    nc.scalar.activation(lrow[:], po[0:1, :N], AF.Copy)
                    pbs = aw.tile([P, N], F32, tag="pbs")
                    nc.gpsimd.partition_broadcast(pbs[:73, :], lrow[:],
                                                  channels=73)
                    nc.vector.reciprocal_approx_fast(pbs[:73, :], pbs[:73, :])
                    nc.vector.tensor_mul(ost[:73, h, :], po[:73, :N],
                                         pbs[:73, :])
                    nc.vector.tensor_add(
                        ost[:73, h, :], ost[:73, h, :],
                        vbs[:73, h, w:w + 1].to_broadcast((73, N)))
                # repack head-slot O into dense feature-major od
                for pc in range(KC):
                    dn = accp.tile([P, 512], F32, tag="acc")
                    pcs = pc_pieces[pc]
                    for i, (idx, h, col0, r0, ln) in enumerate(pcs):
                        nc.tensor.matmul(
                            dn[:, :N], perm_sb[:73, idx * P:(idx + 1) * P],
                            ost[:73, h, :],
                            start=(i == 0), stop=(i == len(pcs) - 1))
                    nc.scalar.activation(od[:, pc, :], dn[:, :N], AF.Copy)
                # dense proj + residual -> xpd / xpdb
                for pc in range(KC):
                    yps = accp.tile([P, 512], F32, tag="acc")
                    for k in range(KC):
                        nc.tensor.matmul(
                            yps[:, :N], wpd_sb[:, (pc * KC + k) * P:
                                               (pc * KC + k + 1) * P],
                            od[:, k, :], start=(k == 0), stop=(k == KC - 1))
                    nc.vector.tensor_add(xw[:, pc, :], xw[:, pc, :],
                                         yps[:, :N])
                    xpo = aw3.tile([P, N], F32, tag="xpo")
                    nc.scalar.activation(xpo[:], xw[:, pc, :], AF.Identity,
                                         bias=b2s[:, pc:pc + 1])
                    xpob = aw3.tile([P, N], BF16, tag="xpob")
                    nc.scalar.activation(xpob[:], xw[:, pc, :], AF.Identity,
                                         bias=b2s[:, pc:pc + 1])
                    nc.sync.dma_start(xpd[w, :, pc, :], xpo[:])
                    nc.sync.dma_start(xpdb[w, :, pc, :], xpob[:])
                if w + 1 < nw:
                    xw = xw2

        # ================= phase C: MLP (pair-granular) ===============
        NPAIR = nw // 2
        KH = 10 if FP8_FC1 else KC
        HDT = FP8 if FP8_FC1 else BF16
        H2DT = FP8 if FP8_FC2 else BF16
        with tc.tile_pool(name="mlp", bufs=1) as mp, \
             tc.tile_pool(name="mw", bufs=2) as mw, \
             tc.tile_pool(name="mw3", bufs=3) as mw3, \
             tc.tile_pool(name="mrows", bufs=1) as mrows, \
             tc.tile_pool(name="mstp", bufs=2, space="PSUM") as mstp, \
             tc.tile_pool(name="maccp", bufs=6, space="PSUM") as maccp:

            if FP8_FC1:
                w1_sb = mp.tile([P, M1T * KP1 * 2 * P], FP8, tag="w1")
                for m in range(M1T):
                    s = slice(m * KP1 * 2 * P, (m + 1) * KP1 * 2 * P)
                    nc.sync.dma_start(w1_sb[:, s], w1[:, s])
            else:
                w1_sb = mp.tile([P, M1T * KC * P], BF16, tag="w1")
                for m in range(M1T):
                    s = slice(m * KC * P, (m + 1) * KC * P)
                    nc.sync.dma_start(w1_sb[:, s], w1[:, s])
            if FP8_FC2:
                w2_sb = mp.tile([P, KC * KP2 * 2 * P], FP8, tag="w2")
                for m in range(KC):
                    s = slice(m * KP2 * 2 * P, (m + 1) * KP2 * 2 * P)
                    nc.sync.dma_start(w2_sb[:, s], w2[:, s])

            h2a = mp.tile([P, M1T, W2N], H2DT, tag="h2a")

            def fetch_xq(pr):
                xq = mw.tile([P, KC, 2, N], F32, tag="xq")
                xqb = mw.tile([P, KC, 2, N], BF16, tag="xqb")
                for s in range(KC):
                    for u in range(2):
                        nc.sync.dma_start(xq[:, s, u, :],
                                          xpd[2 * pr + u, :, s, :])
                        nc.sync.dma_start(xqb[:, s, u, :],
                                          xpdb[2 * pr + u, :, s, :])
                return xq, xqb

            def ln2(xqb):
                xqbf = xqb.rearrange("p s u n -> p s (u n)")
                ms0 = mstp.tile([P, 512], F32, tag="st")
                ms1 = mstp.tile([P, 512], F32, tag="st")
                for s in range(KC):
                    xsq = mw.tile([P, W2N], BF16, tag="mxsq")
                    nc.vector.tensor_mul(xsq[:], xqbf[:, s, :], xqbf[:, s, :])
                    nc.tensor.matmul(ms0[:1, :], ones_c[:], xqbf[:, s, :],
                                     start=(s == 0), stop=(s == KC - 1))
                    nc.tensor.matmul(ms1[:1, :], ones_c[:], xsq[:],
                                     start=(s == 0), stop=(s == KC - 1))
                mean = mrows.tile([1, W2N], F32, tag="m_mean")
                ra = mrows.tile([1, W2N], F32, tag="m_a")
                rb = mrows.tile([1, W2N], F32, tag="m_b")
                nc.vector.tensor_scalar_mul(mean[:], ms0[:1, :], 1.0 / C)
                nc.vector.tensor_scalar_mul(ra[:], ms1[:1, :], 1.0 / C)
                nc.vector.tensor_mul(rb[:], mean[:], mean[:])
                nc.vector.tensor_sub(ra[:], ra[:], rb[:])
                nc.scalar.activation(rb[:], ra[:], AF.Sqrt, bias=EPS)
                nc.vector.reciprocal_approx_fast(ra[:], rb[:])
                rstd = mrows.tile([1, W2N], BF16, tag="m_rstd")
                nc.gpsimd.tensor_copy(rstd[:], ra[:])
                bneg = mrows.tile([1, W2N], BF16, tag="m_bneg")
                nc.vector.scalar_tensor_tensor(
                    bneg[:], mean[:], -1.0, rstd[:],
                    mybir.AluOpType.mult, mybir.AluOpType.mult)
                bc = mstp.tile([P, 512], F32, tag="st")
                nc.tensor.matmul(bc[:], ones_b[:1, :P], rstd[:],
                                 start=True, stop=True)
                bb = mstp.tile([P, 512], F32, tag="st")
                nc.tensor.matmul(bb[:], ones_b[:1, :P], bneg[:],
                                 start=True, stop=True)
                hp = mw.tile([P, KH, W2N], HDT, tag="hp")
                hp_lo = None
                if FP8_FC1 and FC1_COMP:
                    hp_lo = mw.tile([P, KH, W2N], FP8, tag="hplo")
                    nc.vector.memset(hp_lo[:, KC:, :], 0.0)
                if KH > KC:
                    nc.vector.memset(hp[:, KC:, :], 0.0)
                for s in range(KC):
                    if HDT == BF16:
                        nc.vector.tensor_mul(hp[:, s, :], xqbf[:, s, :], bc[:])
                        nc.vector.tensor_add(hp[:, s, :], hp[:, s, :], bb[:])
                    else:
                        th = mw.tile([P, W2N], BF16, tag="mth")
                        nc.vector.tensor_mul(th[:], xqbf[:, s, :], bc[:])
                        nc.vector.tensor_add(th[:], th[:], bb[:])
                        nc.vector.tensor_copy(hp[:, s, :], th[:])
                        if FC1_COMP:
                            terr = mw.tile([P, W2N], BF16, tag="mterr")
                            nc.vector.tensor_sub(terr[:], th[:], hp[:, s, :])
                            nc.vector.tensor_copy(hp_lo[:, s, :], terr[:])
                return hp, hp_lo

            xq, xqb = fetch_xq(0)
            hp, hp_lo = ln2(xqb)
            for pr in range(NPAIR):
                # fc1 -> gelu -> h2a
                for m1 in range(M1T):
                    ps1 = maccp.tile([P, 512], F32, tag="acc")
                    if FP8_FC1:
                        hps = [hp, hp_lo] if FC1_COMP else [hp]
                        for pi, hh in enumerate(hps):
                            for i in range(KP1):
                                lw = w1_sb[:, (m1 * KP1 + i) * 2 * P:
                                           (m1 * KP1 + i + 1) * 2 * P] \
                                    .rearrange("p (j c) -> p j c", j=2)
                                nc.tensor.matmul(
                                    ps1[:], lw, hh[:, 2 * i:2 * i + 2, :],
                                    start=(pi == 0 and i == 0),
                                    stop=(pi == len(hps) - 1 and i == KP1 - 1),
                                    perf_mode=DR)
                        gsc = 1.0 / SW1
                    else:
                        for k in range(KC):
                            nc.tensor.matmul(
                                ps1[:], w1_sb[:, (m1 * KC + k) * P:
                                              (m1 * KC + k + 1) * P],
                                hp[:, k, :], start=(k == 0),
                                stop=(k == KC - 1))
                        gsc = 1.0
                    nc.scalar.activation(h2a[:, m1, :], ps1[:],
                                         AF.Gelu_apprx_tanh,
                                         bias=f1bs[:, m1:m1 + 1], scale=gsc)
                # hoisted LN2 for pr+1 (overlaps fc2)
                if pr + 1 < NPAIR:
                    xq2, xqb2 = fetch_xq(pr + 1)
                    hp, hp_lo = ln2(xqb2)
                # fc2 + residual + output
                for pm in range(KC):
                    ps2 = maccp.tile([P, 512], F32, tag="acc")
                    if FP8_FC2:
                        for i in range(KP2):
                            lw = w2_sb[:, (pm * KP2 + i) * 2 * P:
                                       (pm * KP2 + i + 1) * 2 * P] \
                                .rearrange("p (j c) -> p j c", j=2)
                            nc.tensor.matmul(ps2[:], lw,
                                             h2a[:, 2 * i:2 * i + 2, :],
                                             start=(i == 0),
                                             stop=(i == KP2 - 1),
                                             perf_mode=DR)
                        osc = 1.0 / SW2
                    else:
                        w2t = mw.tile([P, M1T, P], BF16, tag="w2t")
                        nc.sync.dma_start(w2t[:], w2[pm])
                        for m1 in range(M1T):
                            nc.tensor.matmul(ps2[:], w2t[:, m1, :],
                                             h2a[:, m1, :],
                                             start=(m1 == 0),
                                             stop=(m1 == M1T - 1))
                        osc = 1.0
                    tb = mw3.tile([P, W2N], F32, tag="tb")
                    nc.scalar.activation(tb[:], ps2[:], AF.Identity,
                                         bias=b2s[:, KC + pm:KC + pm + 1],
                                         scale=osc)
                    ot = mw3.tile([P, 2, N], F32, tag="ot")
                    nc.vector.tensor_add(
                        ot[:], xq[:, pm, :, :],
                        tb[:].rearrange("p (u n) -> p u n", n=N))
                    for u in range(2):
                        nc.sync.dma_start(outT[2 * pr + u, :, pm, :],
                                          ot[:, u, :])
                if pr + 1 < NPAIR:
                    xq = xq2

    nc.compile()
    return nc


# ---------------------------------------------------------------------------
# host side
# ---------------------------------------------------------------------------

def _qk_colmap():
    m = np.full(2 * H * HS, -1, np.int64)
    for h in range(H):
        m[HS * h:HS * h + DH] = np.arange(DH * h, DH * h + DH)
        m[H * HS + HS * h:H * HS + HS * h + DH] = \
            np.arange(C + DH * h, C + DH * h + DH)
    return m


def _prep_core_inputs(x_c, c_c, wdict):
    """x_c: [nw, N, C], c_c: [nw, C] -> per-core input map"""
    nw = x_c.shape[0]
    xT = np.ascontiguousarray(
        x_c.transpose(0, 2, 1).reshape(nw, KC, P, N).transpose(
            0, 2, 1, 3)).astype(np.float32)
    caug = np.zeros((nw, 1280), np.float32)
    caug[:, :C] = c_c
    caug[:, C] = 1.0
    cT = np.ascontiguousarray(caug.T.reshape(10, P, nw)).astype(NPBF16)
    return {"xT": xT, "xTb": xT.astype(NPBF16), "cT": cT, **wdict}


def _prep_weights(qkv_w, qkv_b, qkvt_w, qkvt_b, rpb_table, rel_idx,
                  proj_w, proj_b, fc1_w, fc1_b, fc2_w, fc2_b):
    qkmap = _qk_colmap()
    valid = qkmap >= 0

    wct = np.zeros((1280, 3072), np.float32)
    wct[:C, valid] = qkvt_w[qkmap[valid], :].T
    wct[C, valid] = (qkv_b + qkvt_b)[qkmap[valid]]
    wct = wct.reshape(10, P, 3072).astype(NPBF16)

    wcv = np.zeros((1280, H, 73), np.float32)
    vmap = np.arange(2 * C, 3 * C).reshape(H, DH)
    for h in range(H):
        wcv[:C, h, 1:73] = qkvt_w[vmap[h], :].T
        wcv[C, h, 1:73] = (qkv_b + qkvt_b)[vmap[h]]
    wcv = wcv.reshape(10, P, H, 73).astype(NPBF16)

    nqk = 2 * H * HS
    wqkT = np.zeros((C, nqk), np.float32)
    wqkT[:, valid[:nqk]] = qkv_w[qkmap[valid[:nqk]], :].T
    # flat [P, QKM*KC*P]: tile (m, k)[p, j] = wqkT[k*P+p, m*P+j]
    wqk = np.ascontiguousarray(
        wqkT.reshape(KC, P, QKM, P).transpose(1, 2, 0, 3).reshape(
            P, QKM * KC * P)).astype(NPBF16)

    # flat [P, 4*KC*288]: tile (nch, k)[p, j] = wvT[k*P+p, nch*288+j]
    wvT = qkv_w[2 * C:, :].T
    wv = np.ascontiguousarray(
        wvT.reshape(KC, P, 4, 288).transpose(1, 2, 0, 3).reshape(
            P, 4 * KC * 288)).astype(NPBF16)

    bias = rpb_table[rel_idx]                      # [N(n), N(m), H]
    expb = np.ascontiguousarray(
        np.exp(bias).transpose(2, 1, 0).reshape(H, 2, P, N).transpose(
            0, 2, 1, 3)).astype(NPBF16)

    # dense proj flat [P, KC*KC*P]: tile (pc, k)[p, j] = proj_w[pc*P+j, k*P+p]
    wpd = np.zeros((P, KC * KC * P), np.float32)
    pw = proj_w.reshape(KC, P, KC, P)              # [pc, j, k, p]
    for pc in range(KC):
        for k in range(KC):
            wpd[:, (pc * KC + k) * P:(pc * KC + k + 1) * P] = pw[pc, :, k, :].T
    wpd = wpd.astype(NPBF16)

    permf = np.zeros((P, 24 * P), np.float32)
    for idx, (pc, h, col0, r0, ln) in enumerate(_repack_pieces()):
        for d in range(ln):
            permf[r0 + d, idx * P + col0 + d] = 1.0
    perm = permf.astype(NPBF16)

    if FP8_FC1:
        w1s = fc1_w * SW1                          # [MLP, C]
        w1p = np.zeros((M1T, KP1 * 2, P, P), np.float32)  # [m, kk, p, j]
        for m in range(M1T):
            for kk in range(KC):
                w1p[m, kk] = w1s[m * P:(m + 1) * P, kk * P:(kk + 1) * P].T
        w1 = np.ascontiguousarray(
            w1p.transpose(2, 0, 1, 3).reshape(P, M1T * KP1 * 2 * P)) \
            .astype(NPFP8)
    else:
        w1 = np.zeros((P, M1T * KC * P), np.float32)
        f1 = fc1_w.reshape(M1T, P, KC, P)          # [m, j, k, p]
        for m in range(M1T):
            for k in range(KC):
                w1[:, (m * KC + k) * P:(m * KC + k + 1) * P] = f1[m, :, k, :].T
        w1 = w1.astype(NPBF16)

    if FP8_FC2:
        w2s = fc2_w * SW2                          # [C, MLP]
        w2p = np.zeros((KC, KP2 * 2, P, P), np.float32)
        for pm in range(KC):
            for kk in range(M1T):
                w2p[pm, kk] = w2s[pm * P:(pm + 1) * P, kk * P:(kk + 1) * P].T
        w2 = np.ascontiguousarray(
            w2p.transpose(2, 0, 1, 3).reshape(P, KC * KP2 * 2 * P)) \
            .astype(NPFP8)
    else:
        w2 = np.ascontiguousarray(
            fc2_w.T.reshape(M1T, P, KC, P).transpose(2, 1, 0, 3)) \
            .astype(NPBF16)

    f1bv = np.ascontiguousarray(fc1_b.reshape(M1T, P).T).astype(np.float32)
    b2T = np.ascontiguousarray(
        np.concatenate([proj_b.reshape(KC, P), fc2_b.reshape(KC, P)])
        .T).astype(np.float32)

    return {"wct": wct, "wcv": wcv, "wqk": wqk, "wv": wv, "expb": expb,
            "wpd": wpd, "perm": perm, "w1": w1, "w2": w2, "f1b": f1bv,
            "b2T": b2T}


_PROGRAM = None


def kernel(x, c, qkv_w, qkv_b, qkvt_w, qkvt_b, rpb_table, proj_w, proj_b,
           fc1_w, fc1_b, fc2_w, fc2_b, rel_idx, _trace=False):
    global _PROGRAM
    x = np.asarray(x, np.float32)
    c = np.asarray(c, np.float32)
    wdict = _prep_weights(
        np.asarray(qkv_w, np.float32), np.asarray(qkv_b, np.float32),
        np.asarray(qkvt_w, np.float32), np.asarray(qkvt_b, np.float32),
        np.asarray(rpb_table, np.float32), np.asarray(rel_idx),
        np.asarray(proj_w, np.float32), np.asarray(proj_b, np.float32),
        np.asarray(fc1_w, np.float32), np.asarray(fc1_b, np.float32),
        np.asarray(fc2_w, np.float32), np.asarray(fc2_b, np.float32))

    if _PROGRAM is None:
        _PROGRAM = build_program(NW)
    nc = _PROGRAM

    in_maps = []
    for core in range(NCORES):
        sl = slice(core * NW, (core + 1) * NW)
        in_maps.append(_prep_core_inputs(x[sl], c[sl], wdict))

    res = bass_utils.run_bass_kernel_spmd(
        nc, in_maps, core_ids=list(range(NCORES)), trace=_trace)

    out = np.empty((B, N, C), np.float32)
    for core in range(NCORES):
        oT = res.results[core]["outT"]            # [NW, P, KC, N]
        out[core * NW:(core + 1) * NW] = \
            oT.transpose(0, 2, 1, 3).reshape(NW, C, N).transpose(0, 2, 1)
    if _trace:
        return out, res
    return out


# revision 45
# speedup vs baseline: 1.2183x; 1.0856x over previous
"""DiffiT transformer block kernel for 8 Trainium2 NeuronCores.

Data-parallel over the B=64 window axis (8 windows per core). Weight-
resident superphases per core:

  A) conditioning: TT = (c_aug @ W_ct)^T feature-major tiles for the QK
     bias (DVE per-partition broadcast at QKV evacuation), and vbs =
     V-part bias in head-slot rows (folded in AFTER softmax
     normalization -- exact, since sum_m P[n,m] = 1).
  B) attention, window-granular (free dim 256): LN1 -> QKV (96-stride
     packed Q/K, token-major V slots with ones col 72) -> per-head
     scores/softmax/PV -> PE permutation-repack of head-slot O into
     dense feature-major O -> dense proj + residual. All attention
     weights resident in SBUF; exp(rel-pos-bias) streamed per head.
  C) MLP, pair-granular (free dim 512): LN2 -> fc1+gelu -> fc2 +
     residual. fp8e4m3 DoubleRow (2x PE) with weight scale 32 folded
     out at PSUM evacuation; weights resident.

Activations are feature-major ([channel, token]) so every linear
contracts over the SBUF partition axis. LN stats come from bf16 shadows
of the fp32 residual stream via ones-matmuls.
"""

import math
from contextlib import ExitStack

import numpy as np
import ml_dtypes

import concourse.bass as bass
import concourse.mybir as mybir
import concourse.tile as tile
from concourse import bacc
from concourse import bass_utils

F32 = mybir.dt.float32
BF16 = mybir.dt.bfloat16
FP8 = mybir.dt.float8e4
NPBF16 = ml_dtypes.bfloat16
NPFP8 = ml_dtypes.float8_e4m3
AF = mybir.ActivationFunctionType
DR = mybir.MatmulPerfMode.DoubleRow

P = 128
WS = 16
N = 256            # tokens per window
C = 1152           # hidden
H = 16             # heads
DH = 72            # head dim
HS = 96            # head stride in the QK packing (32-aligned, >= DH)
MLP = 4608
EPS = 1e-6
B = 64
NCORES = 8
NW = B // NCORES   # windows per core
KC = C // P        # 9 k-tiles over the hidden dim
QKM = 2 * H * HS // P   # 24 m-tiles over packed Q+K (96-stride)
KOFF = QKM // 2    # first K-side m-tile
M1T = MLP // P     # 36 fc1 row tiles
SCALE = 1.0 / math.sqrt(DH)
W2N = 2 * N

# fp8 MLP config (DoubleRow LDWEIGHTS is not hidden at FD=512, so fp8
# only pays off single-pass; 2-pass compensation is slower than bf16)
FP8_FC1 = False
FC1_COMP = False
FP8_FC2 = False
SW1 = 32.0         # fc1 weight scale
SW2 = 32.0         # fc2 weight scale
KP1 = 5            # fc1 doublerow k-pair tiles (9 k-tiles padded to 10)
KP2 = 18           # fc2 doublerow k-pair tiles (36 k-tiles)


def _qk_pieces(h):
    """32-aligned partition pieces covering head h's 72 rows in the
    96-stride packing: [(subtile, base, length), ...]"""
    start, end = HS * h, HS * h + DH
    out = []
    while start < end:
        sub, base = divmod(start, P)
        ln = min(end - start, P - base)
        if base == 64:
            ln = min(ln, 64)
        elif base in (32, 96):
            ln = min(ln, 32)
        elif base != 0:
            raise AssertionError(base)
        out.append((sub, base, ln))
        start += ln
    return out


def _repack_pieces():
    """(pc, h, col0, r0, ln): dense tile pc cols [col0, col0+ln) take ost
    slot-h rows [r0, r0+ln)  (slot row r = d, dense channel 72h + d)."""
    out = []
    for h in range(H):
        c0, c1 = DH * h, DH * h + DH
        while c0 < c1:
            pc, col0 = divmod(c0, P)
            ln = min(c1 - c0, P - col0)
            out.append((pc, h, col0, 1 + (c0 - DH * h), ln))
            c0 += ln
    return out


def build_program(nw=NW):
    nc = bacc.Bacc("TRN2", target_bir_lowering=False, debug=False,
                   num_devices=NCORES)

    # register the layernorm epsilon as a const AP
    eps_t = nc.alloc_sbuf_tensor("const-eps", [P, 1], F32)
    nc.gpsimd.memset(eps_t.ap(), EPS)
    nc.const_aps.aps[(F32, EPS)] = eps_t.ap()
    nc.all_engine_barrier()

    def din(name, shape, dt):
        return nc.dram_tensor(name, shape, dt, kind="ExternalInput").ap()

    xT = din("xT", [nw, P, KC, N], F32)          # x, feature-major
    xTb = din("xTb", [nw, P, KC, N], BF16)       # bf16 shadow for LN1
    cT = din("cT", [10, P, nw], BF16)            # c augmented with ones row
    wct = din("wct", [10, P, 3072], BF16)        # qkvt^T QK part + bias row
    wcv = din("wcv", [10, P, H, 73], BF16)       # qkvt^T V part, slot rows
    wqk = din("wqk", [P, QKM * KC * P], BF16)    # QK weights, 96-stride flat
    wv = din("wv", [P, 4 * KC * 288], BF16)      # V weights, chunk flat
    expb = din("expb", [H, P, 2, N], BF16)       # exp(rel-pos bias)^T per head
    wpd = din("wpd", [P, KC * KC * P], BF16)     # dense proj^T flat
    perm = din("perm", [P, 24 * P], BF16)        # ost-slot -> dense repack
    f1b = din("f1b", [P, M1T], F32)              # fc1 bias, per-partition
    b2T = din("b2T", [P, 2 * KC], F32)           # proj_b ++ fc2_b tile cols
    if FP8_FC1:
        w1 = din("w1", [P, M1T * KP1 * 2 * P], FP8)
    else:
        w1 = din("w1", [M1T, P, KC, P], BF16)
    if FP8_FC2:
        w2 = din("w2", [P, KC * KP2 * 2 * P], FP8)
    else:
        w2 = din("w2", [KC, P, M1T, P], BF16)
    outT = nc.dram_tensor("outT", [nw, P, KC, N], F32,
                          kind="ExternalOutput").ap()

    rp_pieces = _repack_pieces()
    assert len(rp_pieces) == 24

    with tile.TileContext(nc) as tc, ExitStack() as ctx:
        keep = ctx.enter_context(tc.tile_pool(name="keep", bufs=1))
        dram = ctx.enter_context(tc.tile_pool(name="dram", bufs=1,
                                              space="DRAM"))

        ones_b = keep.tile([1, W2N], BF16, tag="ones_b")
        ones_c = keep.tile([P, 1], BF16, tag="ones_c")
        nc.gpsimd.memset(ones_b[:], 1.0)
        nc.gpsimd.memset(ones_c[:], 1.0)
        f1bs = keep.tile([P, M1T], F32, tag="f1bs")
        nc.sync.dma_start(f1bs[:], f1b[:])
        b2s = keep.tile([P, 2 * KC], F32, tag="b2s")
        nc.sync.dma_start(b2s[:], b2T[:])
        TT = keep.tile([P, QKM, nw], F32, tag="TT")     # QK bias, feat-major
        vbs = keep.tile([P, H, nw], F32, tag="vbs")     # V bias, slot rows

        xpd = dram.tile([nw, P, KC, N], F32)     # x after attention branch

        # ================= phase B scope (incl. conditioning) =========
        with tc.tile_pool(name="att", bufs=1) as att, \
             tc.tile_pool(name="aw", bufs=2) as aw, \
             tc.tile_pool(name="aw1", bufs=1) as aw1, \
             tc.tile_pool(name="aw3", bufs=2) as aw3, \
             tc.tile_pool(name="condw", bufs=2) as condw, \
             tc.tile_pool(name="rows", bufs=1) as rows, \
             tc.tile_pool(name="stp", bufs=2, space="PSUM") as stp, \
             tc.tile_pool(name="accp", bufs=6, space="PSUM") as accp:

            def fetch_x(w):
                xw = aw.tile([P, KC, N], F32, tag="xw")
                xbw = aw.tile([P, KC, N], BF16, tag="xbw")
                nc.sync.dma_start(xw[:], xT[w])
                nc.sync.dma_start(xbw[:], xTb[w])
                return xw, xbw

            def ln_stats(xbw):
                """-> (bc, bb) PSUM broadcast tiles [P, :N]"""
                ms0 = stp.tile([P, 512], F32, tag="st")
                ms1 = stp.tile([P, 512], F32, tag="st")
                for s in range(KC):
                    xsq = aw1.tile([P, N], BF16, tag="xsq")
                    nc.vector.tensor_mul(xsq[:], xbw[:, s, :], xbw[:, s, :])
                    nc.tensor.matmul(ms0[:1, :N], ones_c[:], xbw[:, s, :],
                                     start=(s == 0), stop=(s == KC - 1))
                    nc.tensor.matmul(ms1[:1, :N], ones_c[:], xsq[:],
                                     start=(s == 0), stop=(s == KC - 1))
                mean = rows.tile([1, N], F32, tag="r_mean")
                ra = rows.tile([1, N], F32, tag="r_a")
                rb = rows.tile([1, N], F32, tag="r_b")
                nc.vector.tensor_scalar_mul(mean[:], ms0[:1, :N], 1.0 / C)
                nc.vector.tensor_scalar_mul(ra[:], ms1[:1, :N], 1.0 / C)
                nc.vector.tensor_mul(rb[:], mean[:], mean[:])
                nc.vector.tensor_sub(ra[:], ra[:], rb[:])
                nc.scalar.activation(rb[:], ra[:], AF.Sqrt, bias=EPS)
                nc.vector.reciprocal_approx_fast(ra[:], rb[:])
                rstd = rows.tile([1, N], BF16, tag="r_rstd")
                nc.gpsimd.tensor_copy(rstd[:], ra[:])
                bneg = rows.tile([1, N], BF16, tag="r_bneg")
                nc.vector.scalar_tensor_tensor(
                    bneg[:], mean[:], -1.0, rstd[:],
                    mybir.AluOpType.mult, mybir.AluOpType.mult)
                bc = stp.tile([P, 512], F32, tag="st")
                nc.tensor.matmul(bc[:, :N], ones_b[:1, :P], rstd[:],
                                 start=True, stop=True)
                bb = stp.tile([P, 512], F32, tag="st")
                nc.tensor.matmul(bb[:, :N], ones_b[:1, :P], bneg[:],
                                 start=True, stop=True)
                return bc, bb

            def ln_apply(xbw, bc, bb):
                hw = aw.tile([P, KC, N], BF16, tag="hw")
                for s in range(KC):
                    nc.vector.tensor_mul(hw[:, s, :], xbw[:, s, :], bc[:, :N])
                    nc.vector.tensor_add(hw[:, s, :], hw[:, s, :], bb[:, :N])
                return hw

            # window-0 activations first in the DMA queue
            xw, xbw = fetch_x(0)

            # ---- conditioning: TT (QK bias) + vbs (V bias) -----------
            # streamed in wide chunks on the scalar DMA queue
            caug = keep.tile([P, 10, nw], BF16, tag="caug")
            nc.sync.dma_start(caug[:], cT.rearrange("k p w -> p k w"))
            for ch in range(12):                  # 2 j-tiles per chunk
                wcc = condw.tile([P, 10, 2 * P], BF16, tag="wcc")
                nc.scalar.dma_start(
                    wcc[:], wct[:, :, ch * 2 * P:(ch + 1) * 2 * P]
                    .rearrange("k p j -> p k j"))
                for j2 in range(2):
                    j = ch * 2 + j2
                    tps = accp.tile([P, 512], F32, tag="acc")
                    for k in range(10):
                        nc.tensor.matmul(tps[:, :nw],
                                         wcc[:, k, j2 * P:(j2 + 1) * P],
                                         caug[:, k, :],
                                         start=(k == 0), stop=(k == 9))
                    nc.scalar.activation(TT[:, j, :], tps[:, :nw], AF.Copy)
            for ch in range(8):                   # 2 heads per chunk
                wcc = condw.tile([P, 10, 2 * P], BF16, tag="wcc")
                nc.scalar.dma_start(
                    wcc[:, :, :146], wcv[:, :, 2 * ch:2 * ch + 2, :]
                    .rearrange("k p h d -> p k (h d)"))
                for h2 in range(2):
                    h = ch * 2 + h2
                    vps = accp.tile([P, 512], F32, tag="acc")
                    for k in range(10):
                        nc.tensor.matmul(
                            vps[:73, :nw], wcc[:, k, h2 * 73:(h2 + 1) * 73],
                            caug[:, k, :], start=(k == 0), stop=(k == 9))
                    nc.scalar.activation(vbs[:73, h, :], vps[:73, :nw],
                                         AF.Copy)

            # ---- resident attention weights (vector DMA queue) -------
            wqk_sb = att.tile([P, QKM * KC * P], BF16, tag="wqk")
            for m in range(0, QKM, 4):
                s = slice(m * KC * P, (m + 4) * KC * P)
                nc.gpsimd.dma_start(wqk_sb[:, s], wqk[:, s])
            wv_sb = att.tile([P, 4 * KC * 288], BF16, tag="wv")
            nc.gpsimd.dma_start(wv_sb[:], wv[:])
            wpd_sb = att.tile([P, KC * KC * P], BF16, tag="wpd")
            nc.scalar.dma_start(wpd_sb[:], wpd[:])
            perm_sb = att.tile([P, 24 * P], BF16, tag="perm")
            nc.scalar.dma_start(perm_sb[:], perm[:])

            qkst = att.tile([P, QKM, N], BF16, tag="qkst")
            vsl = att.tile([P, 2, H, 73], BF16, tag="vsl")
            ost = att.tile([P, H, N], BF16, tag="ost")
            od = att.tile([P, KC, N], BF16, tag="od")
            pts = att.tile([P, H, 2, N], BF16, tag="pts")
            nc.vector.memset(vsl[:, :, :, 0:1], 1.0)     # ones col (denom)

            pc_pieces = {}
            for idx, (pc, h, col0, r0, ln) in enumerate(rp_pieces):
                pc_pieces.setdefault(pc, []).append((idx, h, col0, r0, ln))

            bc, bb = ln_stats(xbw)
            hw = ln_apply(xbw, bc, bb)
            for w in range(nw):
                # QKV (Q/K 96-stride packed), bias via DVE broadcast of TT
                for m in range(QKM):
                    qs = accp.tile([P, 512], F32, tag="acc")
                    for k in range(KC):
                        nc.tensor.matmul(
                            qs[:, :N], wqk_sb[:, (m * KC + k) * P:
                                              (m * KC + k + 1) * P],
                            hw[:, k, :], start=(k == 0), stop=(k == KC - 1))
                    nc.vector.tensor_add(
                        qkst[:, m, :], qs[:, :N],
                        TT[:, m, w:w + 1].to_broadcast((P, N)))
                # V token-major into per-head slots (data rows 0..71)
                for nch in range(4):
                    for tch in range(2):
                        vs = accp.tile([P, 512], F32, tag="acc")
                        tsl = slice(tch * P, (tch + 1) * P)
                        for k in range(KC):
                            nc.tensor.matmul(
                                vs[:, :288], hw[:, k, tsl],
                                wv_sb[:, (nch * KC + k) * 288:
                                      (nch * KC + k + 1) * 288],
                                start=(k == 0), stop=(k == KC - 1))
                        nc.scalar.activation(
                            vsl[:, tch, 4 * nch:4 * nch + 4, 1:73],
                            vs[:, :288].rearrange("p (h d) -> p h d", d=72),
                            AF.Copy)
                # scores + softmax for all heads (PE never waits on softmax)
                for h in range(H):
                    ebt = aw3.tile([P, 2, N], BF16, tag="ebt")
                    nc.sync.dma_start(ebt[:], expb[h])
                    pieces = _qk_pieces(h)
                    for ms in range(2):
                        ssp = accp.tile([P, 512], F32, tag="acc")
                        msl = slice(ms * P, (ms + 1) * P)
                        for i, (sub, base, ln) in enumerate(pieces):
                            nc.tensor.matmul(
                                ssp[:, :N],
                                qkst[base:base + ln, KOFF + sub, msl],
                                qkst[base:base + ln, sub, :],
                                start=(i == 0), stop=(i == len(pieces) - 1),
                                tile_position=(base, 0))
                        nc.scalar.activation(pts[:, h, ms, :], ssp[:, :N],
                                             AF.Exp, scale=SCALE)
                        nc.vector.tensor_mul(pts[:, h, ms, :],
                                             pts[:, h, ms, :], ebt[:, ms, :])
                # hoisted LN1 for w+1 (PE filler between phases)
                if w + 1 < nw:
                    xw2, xbw2 = fetch_x(w + 1)
                    bc, bb = ln_stats(xbw2)
                    hw = ln_apply(xbw2, bc, bb)
                # PV for all heads (pt inputs long ready)
                for h in range(H):
                    po = accp.tile([P, 512], F32, tag="acc")
                    for ms in range(2):
                        nc.tensor.matmul(po[:73, :N], vsl[:, ms, h, :],
                                         pts[:, h, ms, :],
                                         start=(ms == 0), stop=(ms == 1))
                    lrow = rows.tile([1, N], F32, tag="lrow")
                    nc.scalar.activation(lrow[:], po[0:1, :N], AF.Copy)
                    pbs = aw1.tile([P, N], F32, tag="pbs")
                    nc.gpsimd.partition_broadcast(pbs[:73, :], lrow[:],
                                                  channels=73)
                    nc.vector.reciprocal_approx_fast(pbs[:73, :], pbs[:73, :])
                    nc.vector.tensor_mul(ost[:73, h, :], po[:73, :N],
                                         pbs[:73, :])
                    nc.vector.tensor_add(
                        ost[:73, h, :], ost[:73, h, :],
                        vbs[:73, h, w:w + 1].to_broadcast((73, N)))
                # repack head-slot O into dense feature-major od
                for pc in range(KC):
                    dn = accp.tile([P, 512], F32, tag="acc")
                    pcs = pc_pieces[pc]
                    for i, (idx, h, col0, r0, ln) in enumerate(pcs):
                        nc.tensor.matmul(
                            dn[:, :N], perm_sb[:73, idx * P:(idx + 1) * P],
                            ost[:73, h, :],
                            start=(i == 0), stop=(i == len(pcs) - 1))
                    nc.scalar.activation(od[:, pc, :], dn[:, :N], AF.Copy)
                # dense proj + residual -> xpd
                for pc in range(KC):
                    yps = accp.tile([P, 512], F32, tag="acc")
                    for k in range(KC):
                        nc.tensor.matmul(
                            yps[:, :N], wpd_sb[:, (pc * KC + k) * P:
                                               (pc * KC + k + 1) * P],
                            od[:, k, :], start=(k == 0), stop=(k == KC - 1))
                    nc.vector.tensor_add(xw[:, pc, :], xw[:, pc, :],
                                         yps[:, :N])
                    xpo = aw3.tile([P, N], F32, tag="xpo")
                    nc.scalar.activation(xpo[:], xw[:, pc, :], AF.Identity,
                                         bias=b2s[:, pc:pc + 1])
                    nc.gpsimd.dma_start(xpd[w, :, pc, :], xpo[:])
                if w + 1 < nw:
                    xw = xw2

        # ================= phase C: MLP (pair-granular) ===============
        NPAIR = nw // 2
        KH = 10 if FP8_FC1 else KC
        HDT = FP8 if FP8_FC1 else BF16
        H2DT = FP8 if FP8_FC2 else BF16
        with tc.tile_pool(name="mlp", bufs=1) as mp, \
             tc.tile_pool(name="mw", bufs=2) as mw, \
             tc.tile_pool(name="mw3", bufs=3) as mw3, \
             tc.tile_pool(name="mrows", bufs=1) as mrows, \
             tc.tile_pool(name="mstp", bufs=2, space="PSUM") as mstp, \
             tc.tile_pool(name="maccp", bufs=6, space="PSUM") as maccp:

            if FP8_FC1:
                w1_sb = mp.tile([P, M1T * KP1 * 2 * P], FP8, tag="w1")
                for m in range(M1T):
                    s = slice(m * KP1 * 2 * P, (m + 1) * KP1 * 2 * P)
                    nc.sync.dma_start(w1_sb[:, s], w1[:, s])
            if FP8_FC2:
                w2_sb = mp.tile([P, KC * KP2 * 2 * P], FP8, tag="w2")
                for m in range(KC):
                    s = slice(m * KP2 * 2 * P, (m + 1) * KP2 * 2 * P)
                    nc.sync.dma_start(w2_sb[:, s], w2[:, s])

            h2a = mp.tile([P, M1T, W2N], H2DT, tag="h2a")

            def fetch_xq(pr):
                xq = mw.tile([P, KC, 2, N], F32, tag="xq")
                nc.sync.dma_start(
                    xq[:], xpd[2 * pr:2 * pr + 2]
                    .rearrange("u p s n -> p s u n"))
                xqb = mw.tile([P, KC, 2, N], BF16, tag="xqb")
                for s in range(KC):
                    nc.vector.tensor_copy(
                        xqb[:, s, :, :].rearrange("p u n -> p (u n)"),
                        xq[:, s, :, :].rearrange("p u n -> p (u n)"))
                return xq, xqb

            def ln2(xqb):
                xqbf = xqb.rearrange("p s u n -> p s (u n)")
                ms0 = mstp.tile([P, 512], F32, tag="st")
                ms1 = mstp.tile([P, 512], F32, tag="st")
                for s in range(KC):
                    xsq = mw.tile([P, W2N], BF16, tag="mxsq")
                    nc.vector.tensor_mul(xsq[:], xqbf[:, s, :], xqbf[:, s, :])
                    nc.tensor.matmul(ms0[:1, :], ones_c[:], xqbf[:, s, :],
                                     start=(s == 0), stop=(s == KC - 1))
                    nc.tensor.matmul(ms1[:1, :], ones_c[:], xsq[:],
                                     start=(s == 0), stop=(s == KC - 1))
                mean = mrows.tile([1, W2N], F32, tag="m_mean")
                ra = mrows.tile([1, W2N], F32, tag="m_a")
                rb = mrows.tile([1, W2N], F32, tag="m_b")
                nc.vector.tensor_scalar_mul(mean[:], ms0[:1, :], 1.0 / C)
                nc.vector.tensor_scalar_mul(ra[:], ms1[:1, :], 1.0 / C)
                nc.vector.tensor_mul(rb[:], mean[:], mean[:])
                nc.vector.tensor_sub(ra[:], ra[:], rb[:])
                nc.scalar.activation(rb[:], ra[:], AF.Sqrt, bias=EPS)
                nc.vector.reciprocal_approx_fast(ra[:], rb[:])
                rstd = mrows.tile([1, W2N], BF16, tag="m_rstd")
                nc.gpsimd.tensor_copy(rstd[:], ra[:])
                bneg = mrows.tile([1, W2N], BF16, tag="m_bneg")
                nc.vector.scalar_tensor_tensor(
                    bneg[:], mean[:], -1.0, rstd[:],
                    mybir.AluOpType.mult, mybir.AluOpType.mult)
                bc = mstp.tile([P, 512], F32, tag="st")
                nc.tensor.matmul(bc[:], ones_b[:1, :P], rstd[:],
                                 start=True, stop=True)
                bb = mstp.tile([P, 512], F32, tag="st")
                nc.tensor.matmul(bb[:], ones_b[:1, :P], bneg[:],
                                 start=True, stop=True)
                hp = mw.tile([P, KH, W2N], HDT, tag="hp")
                hp_lo = None
                if FP8_FC1 and FC1_COMP:
                    hp_lo = mw.tile([P, KH, W2N], FP8, tag="hplo")
                    nc.vector.memset(hp_lo[:, KC:, :], 0.0)
                if KH > KC:
                    nc.vector.memset(hp[:, KC:, :], 0.0)
                for s in range(KC):
                    if HDT == BF16:
                        nc.vector.tensor_mul(hp[:, s, :], xqbf[:, s, :], bc[:])
                        nc.vector.tensor_add(hp[:, s, :], hp[:, s, :], bb[:])
                    else:
                        th = mw.tile([P, W2N], BF16, tag="mth")
                        nc.vector.tensor_mul(th[:], xqbf[:, s, :], bc[:])
                        nc.vector.tensor_add(th[:], th[:], bb[:])
                        nc.vector.tensor_copy(hp[:, s, :], th[:])
                        if FC1_COMP:
                            terr = mw.tile([P, W2N], BF16, tag="mterr")
                            nc.vector.tensor_sub(terr[:], th[:], hp[:, s, :])
                            nc.vector.tensor_copy(hp_lo[:, s, :], terr[:])
                return hp, hp_lo

            xq, xqb = fetch_xq(0)
            hp, hp_lo = ln2(xqb)
            for pr in range(NPAIR):
                # fc1 -> gelu -> h2a
                for m1 in range(M1T):
                    ps1 = maccp.tile([P, 512], F32, tag="acc")
                    if FP8_FC1:
                        hps = [hp, hp_lo] if FC1_COMP else [hp]
                        for pi, hh in enumerate(hps):
                            for i in range(KP1):
                                lw = w1_sb[:, (m1 * KP1 + i) * 2 * P:
                                           (m1 * KP1 + i + 1) * 2 * P] \
                                    .rearrange("p (j c) -> p j c", j=2)
                                nc.tensor.matmul(
                                    ps1[:], lw, hh[:, 2 * i:2 * i + 2, :],
                                    start=(pi == 0 and i == 0),
                                    stop=(pi == len(hps) - 1 and i == KP1 - 1),
                                    perf_mode=DR)
                        gsc = 1.0 / SW1
                    else:
                        w1t = mw.tile([P, KC, P], BF16, tag="w1t")
                        nc.scalar.dma_start(w1t[:], w1[m1])
                        for k in range(KC):
                            nc.tensor.matmul(
                                ps1[:], w1t[:, k, :],
                                hp[:, k, :], start=(k == 0),
                                stop=(k == KC - 1))
                        gsc = 1.0
                    nc.scalar.activation(h2a[:, m1, :], ps1[:],
                                         AF.Gelu_apprx_tanh,
                                         bias=f1bs[:, m1:m1 + 1], scale=gsc)
                # hoisted LN2 for pr+1 (overlaps fc2)
                if pr + 1 < NPAIR:
                    xq2, xqb2 = fetch_xq(pr + 1)
                    hp, hp_lo = ln2(xqb2)
                # fc2 + residual + output
                for pm in range(KC):
                    ps2 = maccp.tile([P, 512], F32, tag="acc")
                    if FP8_FC2:
                        for i in range(KP2):
                            lw = w2_sb[:, (pm * KP2 + i) * 2 * P:
                                       (pm * KP2 + i + 1) * 2 * P] \
                                .rearrange("p (j c) -> p j c", j=2)
                            nc.tensor.matmul(ps2[:], lw,
                                             h2a[:, 2 * i:2 * i + 2, :],
                                             start=(i == 0),
                                             stop=(i == KP2 - 1),
                                             perf_mode=DR)
                        osc = 1.0 / SW2
                    else:
                        w2t = mw.tile([P, M1T, P], BF16, tag="w2t")
                        nc.scalar.dma_start(w2t[:], w2[pm])
                        for m1 in range(M1T):
                            nc.tensor.matmul(ps2[:], w2t[:, m1, :],
                                             h2a[:, m1, :],
                                             start=(m1 == 0),
                                             stop=(m1 == M1T - 1))
                        osc = 1.0
                    tb = mw3.tile([P, W2N], F32, tag="tb")
                    nc.scalar.activation(tb[:], ps2[:], AF.Identity,
                                         bias=b2s[:, KC + pm:KC + pm + 1],
                                         scale=osc)
                    ot = mw3.tile([P, 2, N], F32, tag="ot")
                    nc.vector.tensor_add(
                        ot[:], xq[:, pm, :, :],
                        tb[:].rearrange("p (u n) -> p u n", n=N))
                    for u in range(2):
                        nc.gpsimd.dma_start(outT[2 * pr + u, :, pm, :],
                                            ot[:, u, :])
                if pr + 1 < NPAIR:
                    xq = xq2

    nc.compile()
    return nc


# ---------------------------------------------------------------------------
# host side
# ---------------------------------------------------------------------------

def _qk_colmap():
    m = np.full(2 * H * HS, -1, np.int64)
    for h in range(H):
        m[HS * h:HS * h + DH] = np.arange(DH * h, DH * h + DH)
        m[H * HS + HS * h:H * HS + HS * h + DH] = \
            np.arange(C + DH * h, C + DH * h + DH)
    return m


def _prep_core_inputs(x_c, c_c, wdict):
    """x_c: [nw, N, C], c_c: [nw, C] -> per-core input map"""
    nw = x_c.shape[0]
    xT = np.ascontiguousarray(
        x_c.transpose(0, 2, 1).reshape(nw, KC, P, N).transpose(
            0, 2, 1, 3)).astype(np.float32)
    caug = np.zeros((nw, 1280), np.float32)
    caug[:, :C] = c_c
    caug[:, C] = 1.0
    cT = np.ascontiguousarray(caug.T.reshape(10, P, nw)).astype(NPBF16)
    return {"xT": xT, "xTb": xT.astype(NPBF16), "cT": cT, **wdict}


def _prep_weights(qkv_w, qkv_b, qkvt_w, qkvt_b, rpb_table, rel_idx,
                  proj_w, proj_b, fc1_w, fc1_b, fc2_w, fc2_b):
    qkmap = _qk_colmap()
    valid = qkmap >= 0

    wct = np.zeros((1280, 3072), np.float32)
    wct[:C, valid] = qkvt_w[qkmap[valid], :].T
    wct[C, valid] = (qkv_b + qkvt_b)[qkmap[valid]]
    wct = wct.reshape(10, P, 3072).astype(NPBF16)

    wcv = np.zeros((1280, H, 73), np.float32)
    vmap = np.arange(2 * C, 3 * C).reshape(H, DH)
    for h in range(H):
        wcv[:C, h, 1:73] = qkvt_w[vmap[h], :].T
        wcv[C, h, 1:73] = (qkv_b + qkvt_b)[vmap[h]]
    wcv = wcv.reshape(10, P, H, 73).astype(NPBF16)

    nqk = 2 * H * HS
    wqkT = np.zeros((C, nqk), np.float32)
    wqkT[:, valid[:nqk]] = qkv_w[qkmap[valid[:nqk]], :].T
    # flat [P, QKM*KC*P]: tile (m, k)[p, j] = wqkT[k*P+p, m*P+j]
    wqk = np.ascontiguousarray(
        wqkT.reshape(KC, P, QKM, P).transpose(1, 2, 0, 3).reshape(
            P, QKM * KC * P)).astype(NPBF16)

    # flat [P, 4*KC*288]: tile (nch, k)[p, j] = wvT[k*P+p, nch*288+j]
    wvT = qkv_w[2 * C:, :].T
    wv = np.ascontiguousarray(
        wvT.reshape(KC, P, 4, 288).transpose(1, 2, 0, 3).reshape(
            P, 4 * KC * 288)).astype(NPBF16)

    bias = rpb_table[rel_idx]                      # [N(n), N(m), H]
    expb = np.ascontiguousarray(
        np.exp(bias).transpose(2, 1, 0).reshape(H, 2, P, N).transpose(
            0, 2, 1, 3)).astype(NPBF16)

    # dense proj flat [P, KC*KC*P]: tile (pc, k)[p, j] = proj_w[pc*P+j, k*P+p]
    wpd = np.zeros((P, KC * KC * P), np.float32)
    pw = proj_w.reshape(KC, P, KC, P)              # [pc, j, k, p]
    for pc in range(KC):
        for k in range(KC):
            wpd[:, (pc * KC + k) * P:(pc * KC + k + 1) * P] = pw[pc, :, k, :].T
    wpd = wpd.astype(NPBF16)

    permf = np.zeros((P, 24 * P), np.float32)
    for idx, (pc, h, col0, r0, ln) in enumerate(_repack_pieces()):
        for d in range(ln):
            permf[r0 + d, idx * P + col0 + d] = 1.0
    perm = permf.astype(NPBF16)

    if FP8_FC1:
        w1s = fc1_w * SW1                          # [MLP, C]
        w1p = np.zeros((M1T, KP1 * 2, P, P), np.float32)  # [m, kk, p, j]
        for m in range(M1T):
            for kk in range(KC):
                w1p[m, kk] = w1s[m * P:(m + 1) * P, kk * P:(kk + 1) * P].T
        w1 = np.ascontiguousarray(
            w1p.transpose(2, 0, 1, 3).reshape(P, M1T * KP1 * 2 * P)) \
            .astype(NPFP8)
    else:
        # [m, p, k, j] = fc1_w[m*P+j, k*P+p]
        w1 = np.ascontiguousarray(
            fc1_w.reshape(M1T, P, KC, P).transpose(0, 3, 2, 1)) \
            .astype(NPBF16)

    if FP8_FC2:
        w2s = fc2_w * SW2                          # [C, MLP]
        w2p = np.zeros((KC, KP2 * 2, P, P), np.float32)
        for pm in range(KC):
            for kk in range(M1T):
                w2p[pm, kk] = w2s[pm * P:(pm + 1) * P, kk * P:(kk + 1) * P].T
        w2 = np.ascontiguousarray(
            w2p.transpose(2, 0, 1, 3).reshape(P, KC * KP2 * 2 * P)) \
            .astype(NPFP8)
    else:
        w2 = np.ascontiguousarray(
            fc2_w.T.reshape(M1T, P, KC, P).transpose(2, 1, 0, 3)) \
            .astype(NPBF16)

    f1bv = np.ascontiguousarray(fc1_b.reshape(M1T, P).T).astype(np.float32)
    b2T = np.ascontiguousarray(
        np.concatenate([proj_b.reshape(KC, P), fc2_b.reshape(KC, P)])
        .T).astype(np.float32)

    return {"wct": wct, "wcv": wcv, "wqk": wqk, "wv": wv, "expb": expb,
            "wpd": wpd, "perm": perm, "w1": w1, "w2": w2, "f1b": f1bv,
            "b2T": b2T}


_PROGRAM = None


def kernel(x, c, qkv_w, qkv_b, qkvt_w, qkvt_b, rpb_table, proj_w, proj_b,
           fc1_w, fc1_b, fc2_w, fc2_b, rel_idx, _trace=False):
    global _PROGRAM
    x = np.asarray(x, np.float32)
    c = np.asarray(c, np.float32)
    wdict = _prep_weights(
        np.asarray(qkv_w, np.float32), np.asarray(qkv_b, np.float32),
        np.asarray(qkvt_w, np.float32), np.asarray(qkvt_b, np.float32),
        np.asarray(rpb_table, np.float32), np.asarray(rel_idx),
        np.asarray(proj_w, np.float32), np.asarray(proj_b, np.float32),
        np.asarray(fc1_w, np.float32), np.asarray(fc1_b, np.float32),
        np.asarray(fc2_w, np.float32), np.asarray(fc2_b, np.float32))

    if _PROGRAM is None:
        _PROGRAM = build_program(NW)
    nc = _PROGRAM

    in_maps = []
    for core in range(NCORES):
        sl = slice(core * NW, (core + 1) * NW)
        in_maps.append(_prep_core_inputs(x[sl], c[sl], wdict))

    res = bass_utils.run_bass_kernel_spmd(
        nc, in_maps, core_ids=list(range(NCORES)), trace=_trace)

    out = np.empty((B, N, C), np.float32)
    for core in range(NCORES):
        oT = res.results[core]["outT"]            # [NW, P, KC, N]
        out[core * NW:(core + 1) * NW] = \
            oT.transpose(0, 2, 1, 3).reshape(NW, C, N).transpose(0, 2, 1)
    if _trace:
        return out, res
    return out


# revision 62
# speedup vs baseline: 1.3149x; 1.0793x over previous
"""DiffiT transformer block kernel for 8 Trainium2 NeuronCores.

Data-parallel over the B=64 window axis (8 windows per core). Weight-
resident superphases per core:

  A) conditioning: TT = (c_aug @ W_ct)^T feature-major tiles for the QK
     bias (DVE per-partition broadcast at QKV evacuation), and vbs =
     V-part bias in head-slot rows (folded in AFTER softmax
     normalization -- exact, since sum_m P[n,m] = 1).
  B) attention, window-granular (free dim 256): LN1 -> QKV (96-stride
     packed Q/K, token-major V slots with ones col 72) -> per-head
     scores/softmax/PV -> PE permutation-repack of head-slot O into
     dense feature-major O -> dense proj + residual. All attention
     weights resident in SBUF; exp(rel-pos-bias) streamed per head.
  C) MLP, pair-granular (free dim 512): LN2 -> fc1+gelu -> fc2 +
     residual. fp8e4m3 DoubleRow (2x PE) with weight scale 32 folded
     out at PSUM evacuation; weights resident.

Activations are feature-major ([channel, token]) so every linear
contracts over the SBUF partition axis. LN stats come from bf16 shadows
of the fp32 residual stream via ones-matmuls.
"""

import math
from contextlib import ExitStack

import numpy as np
import ml_dtypes

import concourse.bass as bass
import concourse.mybir as mybir
import concourse.tile as tile
from concourse import bacc
from concourse import bass_utils

F32 = mybir.dt.float32
BF16 = mybir.dt.bfloat16
FP8 = mybir.dt.float8e4
NPBF16 = ml_dtypes.bfloat16
NPFP8 = ml_dtypes.float8_e4m3
AF = mybir.ActivationFunctionType
DR = mybir.MatmulPerfMode.DoubleRow

P = 128
WS = 16
N = 256            # tokens per window
C = 1152           # hidden
H = 16             # heads
DH = 72            # head dim
HS = 96            # head stride in the QK packing (32-aligned, >= DH)
MLP = 4608
EPS = 1e-6
B = 64
NCORES = 8
NW = B // NCORES   # windows per core
KC = C // P        # 9 k-tiles over the hidden dim
QKM = 2 * H * HS // P   # 24 m-tiles over packed Q+K (96-stride)
KOFF = QKM // 2    # first K-side m-tile
M1T = MLP // P     # 36 fc1 row tiles
SCALE = 1.0 / math.sqrt(DH)
W2N = 2 * N

# fp8 MLP config (DoubleRow LDWEIGHTS is not hidden at FD=512, so fp8
# only pays off single-pass; 2-pass compensation is slower than bf16)
FP8_FC1 = False
FC1_COMP = False
FP8_FC2 = False
SW1 = 32.0         # fc1 weight scale
SW2 = 32.0         # fc2 weight scale
KP1 = 5            # fc1 doublerow k-pair tiles (9 k-tiles padded to 10)
KP2 = 18           # fc2 doublerow k-pair tiles (36 k-tiles)


def _qk_pieces(h):
    """32-aligned partition pieces covering head h's 72 rows in the
    96-stride packing: [(subtile, base, length), ...]"""
    start, end = HS * h, HS * h + DH
    out = []
    while start < end:
        sub, base = divmod(start, P)
        ln = min(end - start, P - base)
        if base == 64:
            ln = min(ln, 64)
        elif base in (32, 96):
            ln = min(ln, 32)
        elif base != 0:
            raise AssertionError(base)
        out.append((sub, base, ln))
        start += ln
    return out


def _repack_pieces():
    """(pc, h, col0, r0, ln): dense tile pc cols [col0, col0+ln) take ost
    slot-h rows [r0, r0+ln)  (slot row r = d, dense channel 72h + d)."""
    out = []
    for h in range(H):
        c0, c1 = DH * h, DH * h + DH
        while c0 < c1:
            pc, col0 = divmod(c0, P)
            ln = min(c1 - c0, P - col0)
            out.append((pc, h, col0, 1 + (c0 - DH * h), ln))
            c0 += ln
    return out


def build_program(nw=NW):
    nc = bacc.Bacc("TRN2", target_bir_lowering=False, debug=False,
                   num_devices=NCORES)

    # register the layernorm epsilon as a const AP
    eps_t = nc.alloc_sbuf_tensor("const-eps", [P, 1], F32)
    nc.gpsimd.memset(eps_t.ap(), EPS)
    nc.const_aps.aps[(F32, EPS)] = eps_t.ap()
    nc.all_engine_barrier()

    def din(name, shape, dt):
        return nc.dram_tensor(name, shape, dt, kind="ExternalInput").ap()

    xT = din("xT", [nw, P, KC, N], F32)          # x, feature-major
    xTb = din("xTb", [nw, P, KC, N], BF16)       # bf16 shadow for LN1
    cT = din("cT", [10, P, nw], BF16)            # c augmented with ones row
    wctc = din("wctc", [12, P, 10, 2 * P], BF16)  # qkvt^T QK part, chunked
    wcvc = din("wcvc", [5, P, 10, 2 * P], BF16)   # qkvt^T V part, dense chnk
    wqk = din("wqk", [P, QKM * KC * P], BF16)    # QK weights, 96-stride flat
    wv = din("wv", [P, 4 * KC * 288], BF16)      # V weights, chunk flat
    expb = din("expb", [H, P, 2, N], BF16)       # exp(rel-pos bias)^T per head
    wpd = din("wpd", [P, KC * KC * P], BF16)     # dense proj^T flat
    perm = din("perm", [P, 24 * P], BF16)        # ost-slot -> dense repack
    pind = din("pind", [1, 24 * P], BF16)        # piece row indicators
    f1b = din("f1b", [P, M1T], F32)              # fc1 bias, per-partition
    b2T = din("b2T", [P, 2 * KC], F32)           # proj_b ++ fc2_b tile cols
    if FP8_FC1:
        w1 = din("w1", [P, M1T * KP1 * 2 * P], FP8)
    else:
        w1 = din("w1", [M1T, P, KC, P], BF16)
    if FP8_FC2:
        w2 = din("w2", [P, KC * KP2 * 2 * P], FP8)
    else:
        w2 = din("w2", [KC, P, M1T, P], BF16)
    outT = nc.dram_tensor("outT", [nw, P, KC, N], F32,
                          kind="ExternalOutput").ap()

    rp_pieces = _repack_pieces()
    assert len(rp_pieces) == 24

    with tile.TileContext(nc) as tc, ExitStack() as ctx:
        keep = ctx.enter_context(tc.tile_pool(name="keep", bufs=1))
        dram = ctx.enter_context(tc.tile_pool(name="dram", bufs=1,
                                              space="DRAM"))

        ones_b = keep.tile([1, W2N], BF16, tag="ones_b")
        ones_c = keep.tile([P, 1], BF16, tag="ones_c")
        nc.gpsimd.memset(ones_b[:], 1.0)
        nc.gpsimd.memset(ones_c[:], 1.0)
        f1bs = keep.tile([P, M1T], F32, tag="f1bs")
        nc.sync.dma_start(f1bs[:], f1b[:])
        b2s = keep.tile([P, 2 * KC], F32, tag="b2s")
        nc.sync.dma_start(b2s[:], b2T[:])
        TT = keep.tile([P, QKM, nw], F32, tag="TT")     # QK bias, feat-major
        vbd = keep.tile([P, KC, nw], F32, tag="vbd")    # V bias, dense rows

        xpd = dram.tile([nw, P, KC, N], F32)     # x after attention branch

        # ---- conditioning scope: TT (QK bias) + vbd (dense V bias) ---
        caug = keep.tile([P, 10, nw], BF16, tag="caug")
        nc.sync.dma_start(caug[:], cT.rearrange("k p w -> p k w"))
        with tc.tile_pool(name="condw", bufs=2) as condw, \
             tc.tile_pool(name="condp", bufs=4, space="PSUM") as condp:
            for ch in range(12):                  # 2 j-tiles per chunk
                wcc = condw.tile([P, 10, 2 * P], BF16, tag="wcc")
                nc.sync.dma_start(wcc[:], wctc[ch])
                for j2 in range(2):
                    j = ch * 2 + j2
                    tps = condp.tile([P, 512], F32, tag="acc")
                    for k in range(10):
                        nc.tensor.matmul(tps[:, :nw],
                                         wcc[:, k, j2 * P:(j2 + 1) * P],
                                         caug[:, k, :],
                                         start=(k == 0), stop=(k == 9))
                    nc.scalar.activation(TT[:, j, :], tps[:, :nw], AF.Copy)
            for ch in range(5):                   # 2 dense pc-tiles per chunk
                wcc = condw.tile([P, 10, 2 * P], BF16, tag="wcc")
                nc.sync.dma_start(wcc[:], wcvc[ch])
                for j2 in range(2):
                    pc = ch * 2 + j2
                    if pc >= KC:
                        break
                    vps = condp.tile([P, 512], F32, tag="acc")
                    for k in range(10):
                        nc.tensor.matmul(
                            vps[:, :nw], wcc[:, k, j2 * P:(j2 + 1) * P],
                            caug[:, k, :], start=(k == 0), stop=(k == 9))
                    nc.scalar.activation(vbd[:, pc, :], vps[:, :nw], AF.Copy)

        # ================= phase B scope ==============================
        with tc.tile_pool(name="att", bufs=1) as att, \
             tc.tile_pool(name="aw", bufs=2) as aw, \
             tc.tile_pool(name="aw1", bufs=1) as aw1, \
             tc.tile_pool(name="aw3", bufs=2) as aw3, \
             tc.tile_pool(name="rows", bufs=1) as rows, \
             tc.tile_pool(name="stp", bufs=2, space="PSUM") as stp, \
             tc.tile_pool(name="accp", bufs=6, space="PSUM") as accp:

            def fetch_x(w):
                xw = aw.tile([P, KC, N], F32, tag="xw")
                xbw = aw.tile([P, KC, N], BF16, tag="xbw")
                nc.sync.dma_start(xw[:], xT[w])
                nc.sync.dma_start(xbw[:], xTb[w])
                return xw, xbw

            def ln_stats(xbw):
                """-> (bc, bb) PSUM broadcast tiles [P, :N]"""
                ms0 = stp.tile([P, 512], F32, tag="st")
                ms1 = stp.tile([P, 512], F32, tag="st")
                for s in range(KC):
                    xsq = aw1.tile([P, N], BF16, tag="xsq")
                    nc.vector.tensor_mul(xsq[:], xbw[:, s, :], xbw[:, s, :])
                    nc.tensor.matmul(ms0[:1, :N], ones_c[:], xbw[:, s, :],
                                     start=(s == 0), stop=(s == KC - 1))
                    nc.tensor.matmul(ms1[:1, :N], ones_c[:], xsq[:],
                                     start=(s == 0), stop=(s == KC - 1))
                mean = rows.tile([1, N], F32, tag="r_mean")
                ra = rows.tile([1, N], F32, tag="r_a")
                rb = rows.tile([1, N], F32, tag="r_b")
                nc.vector.tensor_scalar_mul(mean[:], ms0[:1, :N], 1.0 / C)
                nc.vector.tensor_scalar_mul(ra[:], ms1[:1, :N], 1.0 / C)
                nc.vector.tensor_mul(rb[:], mean[:], mean[:])
                nc.vector.tensor_sub(ra[:], ra[:], rb[:])
                nc.scalar.activation(rb[:], ra[:], AF.Sqrt, bias=EPS)
                nc.vector.reciprocal_approx_fast(ra[:], rb[:])
                rstd = rows.tile([1, N], BF16, tag="r_rstd")
                nc.gpsimd.tensor_copy(rstd[:], ra[:])
                bneg = rows.tile([1, N], BF16, tag="r_bneg")
                nc.vector.scalar_tensor_tensor(
                    bneg[:], mean[:], -1.0, rstd[:],
                    mybir.AluOpType.mult, mybir.AluOpType.mult)
                bc = stp.tile([P, 512], F32, tag="st")
                nc.tensor.matmul(bc[:, :N], ones_b[:1, :P], rstd[:],
                                 start=True, stop=True)
                bb = stp.tile([P, 512], F32, tag="st")
                nc.tensor.matmul(bb[:, :N], ones_b[:1, :P], bneg[:],
                                 start=True, stop=True)
                return bc, bb

            def ln_apply(xbw, bc, bb):
                hw = aw.tile([P, KC, N], BF16, tag="hw")
                for s in range(KC):
                    nc.vector.tensor_mul(hw[:, s, :], xbw[:, s, :], bc[:, :N])
                    nc.vector.tensor_add(hw[:, s, :], hw[:, s, :], bb[:, :N])
                return hw

            # window-0 activations first in this scope's DMA order
            xw, xbw = fetch_x(0)

            # ---- resident attention weights (gpsimd DMA queue) -------
            wqk_sb = att.tile([P, QKM * KC * P], BF16, tag="wqk")
            for m in range(0, QKM, 4):
                s = slice(m * KC * P, (m + 4) * KC * P)
                nc.gpsimd.dma_start(wqk_sb[:, s], wqk[:, s])
            wv_sb = att.tile([P, 4 * KC * 288], BF16, tag="wv")
            nc.gpsimd.dma_start(wv_sb[:], wv[:])
            wpd_sb = att.tile([P, KC * KC * P], BF16, tag="wpd")
            nc.scalar.dma_start(wpd_sb[:], wpd[:])
            perm_sb = att.tile([P, 24 * P], BF16, tag="perm")
            nc.scalar.dma_start(perm_sb[:], perm[:])
            pind_sb = att.tile([1, 24 * P], BF16, tag="pind")
            nc.scalar.dma_start(pind_sb[:], pind[:])

            qkst = att.tile([P, QKM, N], BF16, tag="qkst")
            vsl = att.tile([P, 2, H, 73], BF16, tag="vsl")
            osr = att.tile([P, H, N], BF16, tag="osr")   # raw (unnormalized)
            od = att.tile([P, KC, N], BF16, tag="od")
            pts = att.tile([P, H, 2, N], BF16, tag="pts")
            nc.vector.memset(vsl[:, :, :, 0:1], 1.0)     # ones col (denom)

            pc_pieces = {}
            for idx, (pc, h, col0, r0, ln) in enumerate(rp_pieces):
                pc_pieces.setdefault(pc, []).append((idx, h, col0, r0, ln))

            bc, bb = ln_stats(xbw)
            hw = ln_apply(xbw, bc, bb)
            for w in range(nw):
                # QKV (Q/K 96-stride packed), bias via DVE broadcast of TT
                for m in range(QKM):
                    qs = accp.tile([P, 512], F32, tag="acc")
                    for k in range(KC):
                        nc.tensor.matmul(
                            qs[:, :N], wqk_sb[:, (m * KC + k) * P:
                                              (m * KC + k + 1) * P],
                            hw[:, k, :], start=(k == 0), stop=(k == KC - 1))
                    nc.vector.tensor_add(
                        qkst[:, m, :], qs[:, :N],
                        TT[:, m, w:w + 1].to_broadcast((P, N)))
                # V token-major into per-head slots (data rows 0..71)
                for nch in range(4):
                    for tch in range(2):
                        vs = accp.tile([P, 512], F32, tag="acc")
                        tsl = slice(tch * P, (tch + 1) * P)
                        for k in range(KC):
                            nc.tensor.matmul(
                                vs[:, :288], hw[:, k, tsl],
                                wv_sb[:, (nch * KC + k) * 288:
                                      (nch * KC + k + 1) * 288],
                                start=(k == 0), stop=(k == KC - 1))
                        nc.scalar.activation(
                            vsl[:, tch, 4 * nch:4 * nch + 4, 1:73],
                            vs[:, :288].rearrange("p (h d) -> p h d", d=72),
                            AF.Copy)
                # scores + softmax for all heads (PE never waits on softmax)
                for h in range(H):
                    ebt = aw3.tile([P, 2, N], BF16, tag="ebt")
                    nc.sync.dma_start(ebt[:], expb[h])
                    pieces = _qk_pieces(h)
                    for ms in range(2):
                        ssp = accp.tile([P, 512], F32, tag="acc")
                        msl = slice(ms * P, (ms + 1) * P)
                        for i, (sub, base, ln) in enumerate(pieces):
                            nc.tensor.matmul(
                                ssp[:, :N],
                                qkst[base:base + ln, KOFF + sub, msl],
                                qkst[base:base + ln, sub, :],
                                start=(i == 0), stop=(i == len(pieces) - 1),
                                tile_position=(base, 0))
                        nc.scalar.activation(pts[:, h, ms, :], ssp[:, :N],
                                             AF.Exp, scale=SCALE)
                        nc.vector.tensor_mul(pts[:, h, ms, :],
                                             pts[:, h, ms, :], ebt[:, ms, :])
                # hoisted LN1 for w+1 (PE filler between phases)
                if w + 1 < nw:
                    xw2, xbw2 = fetch_x(w + 1)
                    bc, bb = ln_stats(xbw2)
                    hw = ln_apply(xbw2, bc, bb)
                # PV for all heads: evacuate raw slots (incl. denom row 0)
                for h in range(H):
                    po = accp.tile([P, 512], F32, tag="acc")
                    for ms in range(2):
                        nc.tensor.matmul(po[:73, :N], vsl[:, ms, h, :],
                                         pts[:, h, ms, :],
                                         start=(ms == 0), stop=(ms == 1))
                    nc.scalar.activation(osr[:73, h, :], po[:73, :N],
                                         AF.Copy)
                # repack head-slot O into dense; normalize + V bias on DVE;
                # dense denominator map built by K=1 indicator matmuls
                for pc in range(KC):
                    pcs = pc_pieces[pc]
                    ldp = accp.tile([P, 512], F32, tag="acc")
                    for i, (idx, h, col0, r0, ln) in enumerate(pcs):
                        nc.tensor.matmul(
                            ldp[:, :N], pind_sb[0:1, idx * P:(idx + 1) * P],
                            osr[0:1, h, :],
                            start=(i == 0), stop=(i == len(pcs) - 1))
                    ldi = aw1.tile([P, N], F32, tag="ldi")
                    nc.vector.reciprocal_approx_fast(ldi[:], ldp[:, :N])
                    dn = accp.tile([P, 512], F32, tag="acc")
                    for i, (idx, h, col0, r0, ln) in enumerate(pcs):
                        nc.tensor.matmul(
                            dn[:, :N], perm_sb[:73, idx * P:(idx + 1) * P],
                            osr[:73, h, :],
                            start=(i == 0), stop=(i == len(pcs) - 1))
                    nc.vector.tensor_mul(ldi[:], dn[:, :N], ldi[:])
                    nc.vector.tensor_add(
                        od[:, pc, :], ldi[:],
                        vbd[:, pc, w:w + 1].to_broadcast((P, N)))
                # dense proj + residual -> xpd
                for pc in range(KC):
                    yps = accp.tile([P, 512], F32, tag="acc")
                    for k in range(KC):
                        nc.tensor.matmul(
                            yps[:, :N], wpd_sb[:, (pc * KC + k) * P:
                                               (pc * KC + k + 1) * P],
                            od[:, k, :], start=(k == 0), stop=(k == KC - 1))
                    nc.vector.tensor_add(xw[:, pc, :], xw[:, pc, :],
                                         yps[:, :N])
                    xpo = aw3.tile([P, N], F32, tag="xpo")
                    nc.scalar.activation(xpo[:], xw[:, pc, :], AF.Identity,
                                         bias=b2s[:, pc:pc + 1])
                    nc.gpsimd.dma_start(xpd[w, :, pc, :], xpo[:])
                if w + 1 < nw:
                    xw = xw2

        # ================= phase C: MLP (pair-granular) ===============
        NPAIR = nw // 2
        KH = 10 if FP8_FC1 else KC
        HDT = FP8 if FP8_FC1 else BF16
        H2DT = FP8 if FP8_FC2 else BF16
        with tc.tile_pool(name="mlp", bufs=1) as mp, \
             tc.tile_pool(name="mw", bufs=2) as mw, \
             tc.tile_pool(name="mw3", bufs=3) as mw3, \
             tc.tile_pool(name="mrows", bufs=1) as mrows, \
             tc.tile_pool(name="mstp", bufs=2, space="PSUM") as mstp, \
             tc.tile_pool(name="maccp", bufs=6, space="PSUM") as maccp:

            if FP8_FC1:
                w1_sb = mp.tile([P, M1T * KP1 * 2 * P], FP8, tag="w1")
                for m in range(M1T):
                    s = slice(m * KP1 * 2 * P, (m + 1) * KP1 * 2 * P)
                    nc.sync.dma_start(w1_sb[:, s], w1[:, s])
            if FP8_FC2:
                w2_sb = mp.tile([P, KC * KP2 * 2 * P], FP8, tag="w2")
                for m in range(KC):
                    s = slice(m * KP2 * 2 * P, (m + 1) * KP2 * 2 * P)
                    nc.sync.dma_start(w2_sb[:, s], w2[:, s])

            h2a = mp.tile([P, M1T, W2N], H2DT, tag="h2a")

            def fetch_xq(pr):
                xq = mw.tile([P, KC, 2, N], F32, tag="xq")
                nc.sync.dma_start(
                    xq[:], xpd[2 * pr:2 * pr + 2]
                    .rearrange("u p s n -> p s u n"))
                xqb = mw.tile([P, KC, 2, N], BF16, tag="xqb")
                for s in range(KC):
                    nc.vector.tensor_copy(
                        xqb[:, s, :, :].rearrange("p u n -> p (u n)"),
                        xq[:, s, :, :].rearrange("p u n -> p (u n)"))
                return xq, xqb

            def ln2(xqb):
                xqbf = xqb.rearrange("p s u n -> p s (u n)")
                ms0 = mstp.tile([P, 512], F32, tag="st")
                ms1 = mstp.tile([P, 512], F32, tag="st")
                for s in range(KC):
                    xsq = mw.tile([P, W2N], BF16, tag="mxsq")
                    nc.vector.tensor_mul(xsq[:], xqbf[:, s, :], xqbf[:, s, :])
                    nc.tensor.matmul(ms0[:1, :], ones_c[:], xqbf[:, s, :],
                                     start=(s == 0), stop=(s == KC - 1))
                    nc.tensor.matmul(ms1[:1, :], ones_c[:], xsq[:],
                                     start=(s == 0), stop=(s == KC - 1))
                mean = mrows.tile([1, W2N], F32, tag="m_mean")
                ra = mrows.tile([1, W2N], F32, tag="m_a")
                rb = mrows.tile([1, W2N], F32, tag="m_b")
                nc.vector.tensor_scalar_mul(mean[:], ms0[:1, :], 1.0 / C)
                nc.vector.tensor_scalar_mul(ra[:], ms1[:1, :], 1.0 / C)
                nc.vector.tensor_mul(rb[:], mean[:], mean[:])
                nc.vector.tensor_sub(ra[:], ra[:], rb[:])
                nc.scalar.activation(rb[:], ra[:], AF.Sqrt, bias=EPS)
                nc.vector.reciprocal_approx_fast(ra[:], rb[:])
                rstd = mrows.tile([1, W2N], BF16, tag="m_rstd")
                nc.gpsimd.tensor_copy(rstd[:], ra[:])
                bneg = mrows.tile([1, W2N], BF16, tag="m_bneg")
                nc.vector.scalar_tensor_tensor(
                    bneg[:], mean[:], -1.0, rstd[:],
                    mybir.AluOpType.mult, mybir.AluOpType.mult)
                bc = mstp.tile([P, 512], F32, tag="st")
                nc.tensor.matmul(bc[:], ones_b[:1, :P], rstd[:],
                                 start=True, stop=True)
                bb = mstp.tile([P, 512], F32, tag="st")
                nc.tensor.matmul(bb[:], ones_b[:1, :P], bneg[:],
                                 start=True, stop=True)
                hp = mw.tile([P, KH, W2N], HDT, tag="hp")
                hp_lo = None
                if FP8_FC1 and FC1_COMP:
                    hp_lo = mw.tile([P, KH, W2N], FP8, tag="hplo")
                    nc.vector.memset(hp_lo[:, KC:, :], 0.0)
                if KH > KC:
                    nc.vector.memset(hp[:, KC:, :], 0.0)
                for s in range(KC):
                    if HDT == BF16:
                        nc.vector.tensor_mul(hp[:, s, :], xqbf[:, s, :], bc[:])
                        nc.vector.tensor_add(hp[:, s, :], hp[:, s, :], bb[:])
                    else:
                        th = mw.tile([P, W2N], BF16, tag="mth")
                        nc.vector.tensor_mul(th[:], xqbf[:, s, :], bc[:])
                        nc.vector.tensor_add(th[:], th[:], bb[:])
                        nc.vector.tensor_copy(hp[:, s, :], th[:])
                        if FC1_COMP:
                            terr = mw.tile([P, W2N], BF16, tag="mterr")
                            nc.vector.tensor_sub(terr[:], th[:], hp[:, s, :])
                            nc.vector.tensor_copy(hp_lo[:, s, :], terr[:])
                return hp, hp_lo

            xq, xqb = fetch_xq(0)
            hp, hp_lo = ln2(xqb)
            for pr in range(NPAIR):
                # fc1 -> gelu -> h2a
                for m1 in range(M1T):
                    ps1 = maccp.tile([P, 512], F32, tag="acc")
                    if FP8_FC1:
                        hps = [hp, hp_lo] if FC1_COMP else [hp]
                        for pi, hh in enumerate(hps):
                            for i in range(KP1):
                                lw = w1_sb[:, (m1 * KP1 + i) * 2 * P:
                                           (m1 * KP1 + i + 1) * 2 * P] \
                                    .rearrange("p (j c) -> p j c", j=2)
                                nc.tensor.matmul(
                                    ps1[:], lw, hh[:, 2 * i:2 * i + 2, :],
                                    start=(pi == 0 and i == 0),
                                    stop=(pi == len(hps) - 1 and i == KP1 - 1),
                                    perf_mode=DR)
                        gsc = 1.0 / SW1
                    else:
                        w1t = mw.tile([P, KC, P], BF16, tag="w1t")
                        nc.scalar.dma_start(w1t[:], w1[m1])
                        for k in range(KC):
                            nc.tensor.matmul(
                                ps1[:], w1t[:, k, :],
                                hp[:, k, :], start=(k == 0),
                                stop=(k == KC - 1))
                        gsc = 1.0
                    nc.scalar.activation(h2a[:, m1, :], ps1[:],
                                         AF.Gelu_apprx_tanh,
                                         bias=f1bs[:, m1:m1 + 1], scale=gsc)
                # hoisted LN2 for pr+1 (overlaps fc2)
                if pr + 1 < NPAIR:
                    xq2, xqb2 = fetch_xq(pr + 1)
                    hp, hp_lo = ln2(xqb2)
                # fc2 + residual + output
                for pm in range(KC):
                    ps2 = maccp.tile([P, 512], F32, tag="acc")
                    if FP8_FC2:
                        for i in range(KP2):
                            lw = w2_sb[:, (pm * KP2 + i) * 2 * P:
                                       (pm * KP2 + i + 1) * 2 * P] \
                                .rearrange("p (j c) -> p j c", j=2)
                            nc.tensor.matmul(ps2[:], lw,
                                             h2a[:, 2 * i:2 * i + 2, :],
                                             start=(i == 0),
                                             stop=(i == KP2 - 1),
                                             perf_mode=DR)
                        osc = 1.0 / SW2
                    else:
                        w2t = mw.tile([P, M1T, P], BF16, tag="w2t")
                        nc.scalar.dma_start(w2t[:], w2[pm])
                        for m1 in range(M1T):
                            nc.tensor.matmul(ps2[:], w2t[:, m1, :],
                                             h2a[:, m1, :],
                                             start=(m1 == 0),
                                             stop=(m1 == M1T - 1))
                        osc = 1.0
                    tb = mw3.tile([P, W2N], F32, tag="tb")
                    nc.scalar.activation(tb[:], ps2[:], AF.Identity,
                                         bias=b2s[:, KC + pm:KC + pm + 1],
                                         scale=osc)
                    ot = mw3.tile([P, 2, N], F32, tag="ot")
                    nc.vector.tensor_add(
                        ot[:], xq[:, pm, :, :],
                        tb[:].rearrange("p (u n) -> p u n", n=N))
                    for u in range(2):
                        nc.gpsimd.dma_start(outT[2 * pr + u, :, pm, :],
                                            ot[:, u, :])
                if pr + 1 < NPAIR:
                    xq = xq2

    nc.compile()
    return nc


# ---------------------------------------------------------------------------
# host side
# ---------------------------------------------------------------------------

def _qk_colmap():
    m = np.full(2 * H * HS, -1, np.int64)
    for h in range(H):
        m[HS * h:HS * h + DH] = np.arange(DH * h, DH * h + DH)
        m[H * HS + HS * h:H * HS + HS * h + DH] = \
            np.arange(C + DH * h, C + DH * h + DH)
    return m


def _prep_core_inputs(x_c, c_c, wdict):
    """x_c: [nw, N, C], c_c: [nw, C] -> per-core input map"""
    nw = x_c.shape[0]
    xT = np.ascontiguousarray(
        x_c.transpose(0, 2, 1).reshape(nw, KC, P, N).transpose(
            0, 2, 1, 3)).astype(np.float32)
    caug = np.zeros((nw, 1280), np.float32)
    caug[:, :C] = c_c
    caug[:, C] = 1.0
    cT = np.ascontiguousarray(caug.T.reshape(10, P, nw)).astype(NPBF16)
    return {"xT": xT, "xTb": xT.astype(NPBF16), "cT": cT, **wdict}


def _prep_weights(qkv_w, qkv_b, qkvt_w, qkvt_b, rpb_table, rel_idx,
                  proj_w, proj_b, fc1_w, fc1_b, fc2_w, fc2_b):
    qkmap = _qk_colmap()
    valid = qkmap >= 0

    # QK conditioning, chunked: wctc[ch, p, k, j] = wct[k*P+p, ch*256+j]
    wct = np.zeros((1280, 3072), np.float32)
    wct[:C, valid] = qkvt_w[qkmap[valid], :].T
    wct[C, valid] = (qkv_b + qkvt_b)[qkmap[valid]]
    wctc = np.ascontiguousarray(
        wct.reshape(10, P, 12, 2 * P).transpose(2, 1, 0, 3)).astype(NPBF16)

    # V conditioning, dense channels: wcvc[ch, p, k, j] = wcv[k*P+p, ch*256+j]
    wcv = np.zeros((1280, 1280), np.float32)
    wcv[:C, :C] = qkvt_w[2 * C:, :].T
    wcv[C, :C] = (qkv_b + qkvt_b)[2 * C:]
    wcvc = np.ascontiguousarray(
        wcv.reshape(10, P, 5, 2 * P).transpose(2, 1, 0, 3)).astype(NPBF16)

    nqk = 2 * H * HS
    wqkT = np.zeros((C, nqk), np.float32)
    wqkT[:, valid[:nqk]] = qkv_w[qkmap[valid[:nqk]], :].T
    # flat [P, QKM*KC*P]: tile (m, k)[p, j] = wqkT[k*P+p, m*P+j]
    wqk = np.ascontiguousarray(
        wqkT.reshape(KC, P, QKM, P).transpose(1, 2, 0, 3).reshape(
            P, QKM * KC * P)).astype(NPBF16)

    # flat [P, 4*KC*288]: tile (nch, k)[p, j] = wvT[k*P+p, nch*288+j]
    wvT = qkv_w[2 * C:, :].T
    wv = np.ascontiguousarray(
        wvT.reshape(KC, P, 4, 288).transpose(1, 2, 0, 3).reshape(
            P, 4 * KC * 288)).astype(NPBF16)

    bias = rpb_table[rel_idx]                      # [N(n), N(m), H]
    expb = np.ascontiguousarray(
        np.exp(bias).transpose(2, 1, 0).reshape(H, 2, P, N).transpose(
            0, 2, 1, 3)).astype(NPBF16)

    # dense proj flat [P, KC*KC*P]: tile (pc, k)[p, j] = proj_w[pc*P+j, k*P+p]
    wpd = np.zeros((P, KC * KC * P), np.float32)
    pw = proj_w.reshape(KC, P, KC, P)              # [pc, j, k, p]
    for pc in range(KC):
        for k in range(KC):
            wpd[:, (pc * KC + k) * P:(pc * KC + k + 1) * P] = pw[pc, :, k, :].T
    wpd = wpd.astype(NPBF16)

    permf = np.zeros((P, 24 * P), np.float32)
    pindf = np.zeros((1, 24 * P), np.float32)
    for idx, (pc, h, col0, r0, ln) in enumerate(_repack_pieces()):
        for d in range(ln):
            permf[r0 + d, idx * P + col0 + d] = 1.0
        pindf[0, idx * P + col0:idx * P + col0 + ln] = 1.0
    perm = permf.astype(NPBF16)
    pind = pindf.astype(NPBF16)

    if FP8_FC1:
        w1s = fc1_w * SW1                          # [MLP, C]
        w1p = np.zeros((M1T, KP1 * 2, P, P), np.float32)  # [m, kk, p, j]
        for m in range(M1T):
            for kk in range(KC):
                w1p[m, kk] = w1s[m * P:(m + 1) * P, kk * P:(kk + 1) * P].T
        w1 = np.ascontiguousarray(
            w1p.transpose(2, 0, 1, 3).reshape(P, M1T * KP1 * 2 * P)) \
            .astype(NPFP8)
    else:
        # [m, p, k, j] = fc1_w[m*P+j, k*P+p]
        w1 = np.ascontiguousarray(
            fc1_w.reshape(M1T, P, KC, P).transpose(0, 3, 2, 1)) \
            .astype(NPBF16)

    if FP8_FC2:
        w2s = fc2_w * SW2                          # [C, MLP]
        w2p = np.zeros((KC, KP2 * 2, P, P), np.float32)
        for pm in range(KC):
            for kk in range(M1T):
                w2p[pm, kk] = w2s[pm * P:(pm + 1) * P, kk * P:(kk + 1) * P].T
        w2 = np.ascontiguousarray(
            w2p.transpose(2, 0, 1, 3).reshape(P, KC * KP2 * 2 * P)) \
            .astype(NPFP8)
    else:
        w2 = np.ascontiguousarray(
            fc2_w.T.reshape(M1T, P, KC, P).transpose(2, 1, 0, 3)) \
            .astype(NPBF16)

    f1bv = np.ascontiguousarray(fc1_b.reshape(M1T, P).T).astype(np.float32)
    b2T = np.ascontiguousarray(
        np.concatenate([proj_b.reshape(KC, P), fc2_b.reshape(KC, P)])
        .T).astype(np.float32)

    return {"wctc": wctc, "wcvc": wcvc, "wqk": wqk, "wv": wv, "expb": expb,
            "wpd": wpd, "perm": perm, "pind": pind, "w1": w1, "w2": w2,
            "f1b": f1bv, "b2T": b2T}


_PROGRAM = None


def kernel(x, c, qkv_w, qkv_b, qkvt_w, qkvt_b, rpb_table, proj_w, proj_b,
           fc1_w, fc1_b, fc2_w, fc2_b, rel_idx, _trace=False):
    global _PROGRAM
    x = np.asarray(x, np.float32)
    c = np.asarray(c, np.float32)
    wdict = _prep_weights(
        np.asarray(qkv_w, np.float32), np.asarray(qkv_b, np.float32),
        np.asarray(qkvt_w, np.float32), np.asarray(qkvt_b, np.float32),
        np.asarray(rpb_table, np.float32), np.asarray(rel_idx),
        np.asarray(proj_w, np.float32), np.asarray(proj_b, np.float32),
        np.asarray(fc1_w, np.float32), np.asarray(fc1_b, np.float32),
        np.asarray(fc2_w, np.float32), np.asarray(fc2_b, np.float32))

    if _PROGRAM is None:
        _PROGRAM = build_program(NW)
    nc = _PROGRAM

    in_maps = []
    for core in range(NCORES):
        sl = slice(core * NW, (core + 1) * NW)
        in_maps.append(_prep_core_inputs(x[sl], c[sl], wdict))

    res = bass_utils.run_bass_kernel_spmd(
        nc, in_maps, core_ids=list(range(NCORES)), trace=_trace)

    out = np.empty((B, N, C), np.float32)
    for core in range(NCORES):
        oT = res.results[core]["outT"]            # [NW, P, KC, N]
        out[core * NW:(core + 1) * NW] = \
            oT.transpose(0, 2, 1, 3).reshape(NW, C, N).transpose(0, 2, 1)
    if _trace:
        return out, res
    return out


# revision 66
# speedup vs baseline: 1.3878x; 1.0555x over previous
"""DiffiT transformer block kernel for 8 Trainium2 NeuronCores.

Data-parallel over the B=64 window axis (8 windows per core). Weight-
resident superphases per core:

  A) conditioning: TT = (c_aug @ W_ct)^T feature-major tiles for the QK
     bias (DVE per-partition broadcast at QKV evacuation), and vbs =
     V-part bias in head-slot rows (folded in AFTER softmax
     normalization -- exact, since sum_m P[n,m] = 1).
  B) attention, window-granular (free dim 256): LN1 -> QKV (96-stride
     packed Q/K, token-major V slots with ones col 72) -> per-head
     scores/softmax/PV -> PE permutation-repack of head-slot O into
     dense feature-major O -> dense proj + residual. All attention
     weights resident in SBUF; exp(rel-pos-bias) streamed per head.
  C) MLP, pair-granular (free dim 512): LN2 -> fc1+gelu -> fc2 +
     residual. fp8e4m3 DoubleRow (2x PE) with weight scale 32 folded
     out at PSUM evacuation; weights resident.

Activations are feature-major ([channel, token]) so every linear
contracts over the SBUF partition axis. LN stats come from bf16 shadows
of the fp32 residual stream via ones-matmuls.
"""

import math
from contextlib import ExitStack

import numpy as np
import ml_dtypes

import concourse.bass as bass
import concourse.mybir as mybir
import concourse.tile as tile
from concourse import bacc
from concourse import bass_utils

F32 = mybir.dt.float32
BF16 = mybir.dt.bfloat16
FP8 = mybir.dt.float8e4
NPBF16 = ml_dtypes.bfloat16
NPFP8 = ml_dtypes.float8_e4m3
AF = mybir.ActivationFunctionType
DR = mybir.MatmulPerfMode.DoubleRow

P = 128
WS = 16
N = 256            # tokens per window
C = 1152           # hidden
H = 16             # heads
DH = 72            # head dim
HS = 96            # head stride in the QK packing (32-aligned, >= DH)
MLP = 4608
EPS = 1e-6
B = 64
NCORES = 8
NW = B // NCORES   # windows per core
KC = C // P        # 9 k-tiles over the hidden dim
QKM = 2 * H * HS // P   # 24 m-tiles over packed Q+K (96-stride)
KOFF = QKM // 2    # first K-side m-tile
M1T = MLP // P     # 36 fc1 row tiles
SCALE = 1.0 / math.sqrt(DH)
W2N = 2 * N

# fp8 MLP config (DoubleRow LDWEIGHTS is not hidden at FD=512, so fp8
# only pays off single-pass; 2-pass compensation is slower than bf16)
FP8_FC1 = False
FC1_COMP = False
FP8_FC2 = False
SW1 = 32.0         # fc1 weight scale
SW2 = 32.0         # fc2 weight scale
KP1 = 5            # fc1 doublerow k-pair tiles (9 k-tiles padded to 10)
KP2 = 18           # fc2 doublerow k-pair tiles (36 k-tiles)


def _qk_pieces(h):
    """32-aligned partition pieces covering head h's 72 rows in the
    96-stride packing: [(subtile, base, length), ...]"""
    start, end = HS * h, HS * h + DH
    out = []
    while start < end:
        sub, base = divmod(start, P)
        ln = min(end - start, P - base)
        if base == 64:
            ln = min(ln, 64)
        elif base in (32, 96):
            ln = min(ln, 32)
        elif base != 0:
            raise AssertionError(base)
        out.append((sub, base, ln))
        start += ln
    return out


def _repack_pieces():
    """(pc, h, col0, r0, ln): dense tile pc cols [col0, col0+ln) take ost
    slot-h rows [r0, r0+ln)  (slot row r = d, dense channel 72h + d)."""
    out = []
    for h in range(H):
        c0, c1 = DH * h, DH * h + DH
        while c0 < c1:
            pc, col0 = divmod(c0, P)
            ln = min(c1 - c0, P - col0)
            out.append((pc, h, col0, 1 + (c0 - DH * h), ln))
            c0 += ln
    return out


def build_program(nw=NW):
    nc = bacc.Bacc("TRN2", target_bir_lowering=False, debug=False,
                   num_devices=NCORES)

    # register the layernorm epsilon as a const AP
    eps_t = nc.alloc_sbuf_tensor("const-eps", [P, 1], F32)
    nc.gpsimd.memset(eps_t.ap(), EPS)
    nc.const_aps.aps[(F32, EPS)] = eps_t.ap()
    nc.all_engine_barrier()

    def din(name, shape, dt):
        return nc.dram_tensor(name, shape, dt, kind="ExternalInput").ap()

    xT = din("xT", [nw, P, KC, N], F32)          # x, feature-major
    xTb = din("xTb", [nw, P, KC, N], BF16)       # bf16 shadow for LN1
    cT = din("cT", [10, P, nw], BF16)            # c augmented with ones row
    wctc = din("wctc", [12, P, 10, 2 * P], BF16)  # qkvt^T QK part, chunked
    wcvc = din("wcvc", [5, P, 10, 2 * P], BF16)   # qkvt^T V part, dense chnk
    wqk = din("wqk", [P, QKM * KC * P], BF16)    # QK weights, 96-stride flat
    wv = din("wv", [P, 4 * KC * 288], BF16)      # V weights, chunk flat
    expb = din("expb", [H, P, 2, N], BF16)       # exp(rel-pos bias)^T per head
    wpd = din("wpd", [P, KC * KC * P], BF16)     # dense proj^T flat
    perm = din("perm", [P, 24 * P], BF16)        # ost-slot -> dense repack
    pind = din("pind", [1, 24 * P], BF16)        # piece row indicators
    f1b = din("f1b", [P, M1T], F32)              # fc1 bias, per-partition
    b2T = din("b2T", [P, 2 * KC], F32)           # proj_b ++ fc2_b tile cols
    if FP8_FC1:
        w1 = din("w1", [P, M1T * KP1 * 2 * P], FP8)
    else:
        w1 = din("w1", [M1T, P, KC, P], BF16)
    if FP8_FC2:
        w2 = din("w2", [P, KC * KP2 * 2 * P], FP8)
    else:
        w2 = din("w2", [KC, P, M1T, P], BF16)
    outT = nc.dram_tensor("outT", [nw, P, KC, N], F32,
                          kind="ExternalOutput").ap()

    rp_pieces = _repack_pieces()
    assert len(rp_pieces) == 24

    with tile.TileContext(nc) as tc, ExitStack() as ctx:
        keep = ctx.enter_context(tc.tile_pool(name="keep", bufs=1))
        dram = ctx.enter_context(tc.tile_pool(name="dram", bufs=1,
                                              space="DRAM"))

        ones_b = keep.tile([1, W2N], BF16, tag="ones_b")
        ones_c = keep.tile([P, 1], BF16, tag="ones_c")
        nc.gpsimd.memset(ones_b[:], 1.0)
        nc.gpsimd.memset(ones_c[:], 1.0)
        f1bs = keep.tile([P, M1T], F32, tag="f1bs")
        nc.sync.dma_start(f1bs[:], f1b[:])
        b2s = keep.tile([P, 2 * KC], F32, tag="b2s")
        nc.sync.dma_start(b2s[:], b2T[:])
        TT = keep.tile([P, QKM, nw], F32, tag="TT")     # QK bias, feat-major
        vbd = keep.tile([P, KC, nw], F32, tag="vbd")    # V bias, dense rows

        xpd = dram.tile([nw, P, KC, N], F32)     # x after attention branch

        # ---- conditioning scope: TT (QK bias) + vbd (dense V bias) ---
        caug = keep.tile([P, 10, nw], BF16, tag="caug")
        nc.sync.dma_start(caug[:], cT.rearrange("k p w -> p k w"))
        with tc.tile_pool(name="condw", bufs=2) as condw, \
             tc.tile_pool(name="condp", bufs=4, space="PSUM") as condp:
            for ch in range(12):                  # 2 j-tiles per chunk
                wcc = condw.tile([P, 10, 2 * P], BF16, tag="wcc")
                nc.sync.dma_start(wcc[:], wctc[ch])
                for j2 in range(2):
                    j = ch * 2 + j2
                    tps = condp.tile([P, 512], F32, tag="acc")
                    for k in range(10):
                        nc.tensor.matmul(tps[:, :nw],
                                         wcc[:, k, j2 * P:(j2 + 1) * P],
                                         caug[:, k, :],
                                         start=(k == 0), stop=(k == 9))
                    nc.scalar.activation(TT[:, j, :], tps[:, :nw], AF.Copy)
            for ch in range(5):                   # 2 dense pc-tiles per chunk
                wcc = condw.tile([P, 10, 2 * P], BF16, tag="wcc")
                nc.sync.dma_start(wcc[:], wcvc[ch])
                for j2 in range(2):
                    pc = ch * 2 + j2
                    if pc >= KC:
                        break
                    vps = condp.tile([P, 512], F32, tag="acc")
                    for k in range(10):
                        nc.tensor.matmul(
                            vps[:, :nw], wcc[:, k, j2 * P:(j2 + 1) * P],
                            caug[:, k, :], start=(k == 0), stop=(k == 9))
                    nc.scalar.activation(vbd[:, pc, :], vps[:, :nw], AF.Copy)

        # ================= phase B scope ==============================
        with tc.tile_pool(name="att", bufs=1) as att, \
             tc.tile_pool(name="aw", bufs=2) as aw, \
             tc.tile_pool(name="aw1", bufs=1) as aw1, \
             tc.tile_pool(name="aw3", bufs=2) as aw3, \
             tc.tile_pool(name="rows", bufs=1) as rows, \
             tc.tile_pool(name="stp", bufs=2, space="PSUM") as stp, \
             tc.tile_pool(name="accp", bufs=6, space="PSUM") as accp:

            def fetch_x(w):
                xw = aw.tile([P, KC, N], F32, tag="xw")
                xbw = aw.tile([P, KC, N], BF16, tag="xbw")
                nc.sync.dma_start(xw[:], xT[w])
                nc.sync.dma_start(xbw[:], xTb[w])
                return xw, xbw

            def ln_stats(xbw):
                """-> (bc, bb) PSUM broadcast tiles [P, :N]"""
                ms0 = stp.tile([P, 512], F32, tag="st")
                ms1 = stp.tile([P, 512], F32, tag="st")
                for s in range(KC):
                    xsq = aw1.tile([P, N], BF16, tag="xsq")
                    nc.vector.tensor_mul(xsq[:], xbw[:, s, :], xbw[:, s, :])
                    nc.tensor.matmul(ms0[:1, :N], ones_c[:], xbw[:, s, :],
                                     start=(s == 0), stop=(s == KC - 1))
                    nc.tensor.matmul(ms1[:1, :N], ones_c[:], xsq[:],
                                     start=(s == 0), stop=(s == KC - 1))
                mean = rows.tile([1, N], F32, tag="r_mean")
                ra = rows.tile([1, N], F32, tag="r_a")
                rb = rows.tile([1, N], F32, tag="r_b")
                nc.vector.tensor_scalar_mul(mean[:], ms0[:1, :N], 1.0 / C)
                nc.vector.tensor_scalar_mul(ra[:], ms1[:1, :N], 1.0 / C)
                nc.vector.tensor_mul(rb[:], mean[:], mean[:])
                nc.vector.tensor_sub(ra[:], ra[:], rb[:])
                nc.scalar.activation(rb[:], ra[:], AF.Sqrt, bias=EPS)
                nc.vector.reciprocal_approx_fast(ra[:], rb[:])
                rstd = rows.tile([1, N], BF16, tag="r_rstd")
                nc.gpsimd.tensor_copy(rstd[:], ra[:])
                bneg = rows.tile([1, N], BF16, tag="r_bneg")
                nc.vector.scalar_tensor_tensor(
                    bneg[:], mean[:], -1.0, rstd[:],
                    mybir.AluOpType.mult, mybir.AluOpType.mult)
                bc = stp.tile([P, 512], F32, tag="st")
                nc.tensor.matmul(bc[:, :N], ones_b[:1, :P], rstd[:],
                                 start=True, stop=True)
                bb = stp.tile([P, 512], F32, tag="st")
                nc.tensor.matmul(bb[:, :N], ones_b[:1, :P], bneg[:],
                                 start=True, stop=True)
                return bc, bb

            def ln_apply(xbw, bc, bb):
                hw = aw.tile([P, KC, N], BF16, tag="hw")
                for s in range(KC):
                    nc.vector.tensor_mul(hw[:, s, :], xbw[:, s, :], bc[:, :N])
                    nc.vector.tensor_add(hw[:, s, :], hw[:, s, :], bb[:, :N])
                return hw

            # window-0 activations first in this scope's DMA order
            xw, xbw = fetch_x(0)

            # ---- resident attention weights (gpsimd DMA queue) -------
            wqk_sb = att.tile([P, QKM * KC * P], BF16, tag="wqk")
            for m in range(0, QKM, 4):
                s = slice(m * KC * P, (m + 4) * KC * P)
                nc.gpsimd.dma_start(wqk_sb[:, s], wqk[:, s])
            wv_sb = att.tile([P, 4 * KC * 288], BF16, tag="wv")
            nc.gpsimd.dma_start(wv_sb[:], wv[:])
            wpd_sb = att.tile([P, KC * KC * P], BF16, tag="wpd")
            nc.scalar.dma_start(wpd_sb[:], wpd[:])
            perm_sb = att.tile([P, 24 * P], BF16, tag="perm")
            nc.scalar.dma_start(perm_sb[:], perm[:])
            pind_sb = att.tile([1, 24 * P], BF16, tag="pind")
            nc.scalar.dma_start(pind_sb[:], pind[:])

            qkst = att.tile([P, QKM, N], BF16, tag="qkst")
            vsl = att.tile([P, 2, H, 73], BF16, tag="vsl")
            osr = att.tile([P, H, N], BF16, tag="osr")   # raw (unnormalized)
            od = att.tile([P, KC, N], BF16, tag="od")
            pts = att.tile([P, H, 2, N], BF16, tag="pts")
            nc.vector.memset(vsl[:, :, :, 0:1], 1.0)     # ones col (denom)

            pc_pieces = {}
            for idx, (pc, h, col0, r0, ln) in enumerate(rp_pieces):
                pc_pieces.setdefault(pc, []).append((idx, h, col0, r0, ln))

            bc, bb = ln_stats(xbw)
            hw = ln_apply(xbw, bc, bb)
            for w in range(nw):
                # QKV (Q/K 96-stride packed), bias via DVE broadcast of TT
                for m in range(QKM):
                    qs = accp.tile([P, 512], F32, tag="acc")
                    for k in range(KC):
                        nc.tensor.matmul(
                            qs[:, :N], wqk_sb[:, (m * KC + k) * P:
                                              (m * KC + k + 1) * P],
                            hw[:, k, :], start=(k == 0), stop=(k == KC - 1))
                    nc.vector.tensor_add(
                        qkst[:, m, :], qs[:, :N],
                        TT[:, m, w:w + 1].to_broadcast((P, N)))
                # V token-major into per-head slots (data rows 0..71)
                for nch in range(4):
                    for tch in range(2):
                        vs = accp.tile([P, 512], F32, tag="acc")
                        tsl = slice(tch * P, (tch + 1) * P)
                        for k in range(KC):
                            nc.tensor.matmul(
                                vs[:, :288], hw[:, k, tsl],
                                wv_sb[:, (nch * KC + k) * 288:
                                      (nch * KC + k + 1) * 288],
                                start=(k == 0), stop=(k == KC - 1))
                        nc.scalar.activation(
                            vsl[:, tch, 4 * nch:4 * nch + 4, 1:73],
                            vs[:, :288].rearrange("p (h d) -> p h d", d=72),
                            AF.Copy)
                # scores + softmax for all heads (PE never waits on softmax)
                for h in range(H):
                    ebt = aw3.tile([P, 2, N], BF16, tag="ebt")
                    nc.sync.dma_start(ebt[:], expb[h])
                    pieces = _qk_pieces(h)
                    ssp = accp.tile([P, 512], F32, tag="acc")
                    for ms in range(2):
                        msl = slice(ms * P, (ms + 1) * P)
                        for i, (sub, base, ln) in enumerate(pieces):
                            nc.tensor.matmul(
                                ssp[:, ms * N:(ms + 1) * N],
                                qkst[base:base + ln, KOFF + sub, msl],
                                qkst[base:base + ln, sub, :],
                                start=(ms == 0 and i == 0),
                                stop=(ms == 1 and i == len(pieces) - 1),
                                tile_position=(base, 0))
                    ptf = pts[:, h, :, :].rearrange("p u n -> p (u n)")
                    nc.scalar.activation(ptf, ssp[:], AF.Exp, scale=SCALE)
                    nc.vector.tensor_mul(
                        ptf, ptf, ebt[:].rearrange("p u n -> p (u n)"))
                # hoisted LN1 for w+1 (PE filler between phases)
                if w + 1 < nw:
                    xw2, xbw2 = fetch_x(w + 1)
                    bc, bb = ln_stats(xbw2)
                    hw = ln_apply(xbw2, bc, bb)
                # PV for all heads: evacuate raw slots (incl. denom row 0),
                # alternating ACT/DVE so neither queue throttles the PE
                for h in range(H):
                    po = accp.tile([P, 512], F32, tag="acc")
                    for ms in range(2):
                        nc.tensor.matmul(po[:73, :N], vsl[:, ms, h, :],
                                         pts[:, h, ms, :],
                                         start=(ms == 0), stop=(ms == 1))
                    if h % 2 == 0:
                        nc.scalar.activation(osr[:73, h, :], po[:73, :N],
                                             AF.Copy)
                    else:
                        nc.vector.tensor_copy(osr[:73, h, :], po[:73, :N])
                # repack head-slot O into dense; normalize + V bias on DVE;
                # dense denominator map built by K=1 indicator matmuls
                for pc in range(KC):
                    pcs = pc_pieces[pc]
                    ldp = accp.tile([P, 512], F32, tag="acc")
                    for i, (idx, h, col0, r0, ln) in enumerate(pcs):
                        nc.tensor.matmul(
                            ldp[:, :N], pind_sb[0:1, idx * P:(idx + 1) * P],
                            osr[0:1, h, :],
                            start=(i == 0), stop=(i == len(pcs) - 1))
                    ldi = aw3.tile([P, N], F32, tag="ldi")
                    nc.vector.reciprocal_approx_fast(ldi[:], ldp[:, :N])
                    dn = accp.tile([P, 512], F32, tag="acc")
                    for i, (idx, h, col0, r0, ln) in enumerate(pcs):
                        nc.tensor.matmul(
                            dn[:, :N], perm_sb[:73, idx * P:(idx + 1) * P],
                            osr[:73, h, :],
                            start=(i == 0), stop=(i == len(pcs) - 1))
                    nc.vector.tensor_mul(ldi[:], dn[:, :N], ldi[:])
                    nc.vector.tensor_add(
                        od[:, pc, :], ldi[:],
                        vbd[:, pc, w:w + 1].to_broadcast((P, N)))
                # dense proj + residual -> xpd (k-outer over pc-groups so
                # the PE starts as soon as od k-tile 0 is ready)
                for pg in range(3):
                    ypss = [accp.tile([P, 512], F32, tag="acc")
                            for _ in range(3)]
                    for k in range(KC):
                        for i in range(3):
                            pc = 3 * pg + i
                            nc.tensor.matmul(
                                ypss[i][:, :N],
                                wpd_sb[:, (pc * KC + k) * P:
                                       (pc * KC + k + 1) * P],
                                od[:, k, :], start=(k == 0),
                                stop=(k == KC - 1))
                    for i in range(3):
                        pc = 3 * pg + i
                        nc.vector.tensor_add(xw[:, pc, :], xw[:, pc, :],
                                             ypss[i][:, :N])
                        xpo = aw3.tile([P, N], F32, tag="xpo")
                        nc.scalar.activation(xpo[:], xw[:, pc, :],
                                             AF.Identity,
                                             bias=b2s[:, pc:pc + 1])
                        nc.gpsimd.dma_start(xpd[w, :, pc, :], xpo[:])
                if w + 1 < nw:
                    xw = xw2

        # ===== phase C: MLP, 2-pair super-steps (weights feed 2 groups) ===
        NPAIR = nw // 2
        with tc.tile_pool(name="mlp", bufs=1) as mp, \
             tc.tile_pool(name="mw", bufs=2) as mw, \
             tc.tile_pool(name="mw1", bufs=3) as mw1, \
             tc.tile_pool(name="mw3", bufs=3) as mw3, \
             tc.tile_pool(name="mrows", bufs=1) as mrows, \
             tc.tile_pool(name="mstp", bufs=2, space="PSUM") as mstp, \
             tc.tile_pool(name="maccp", bufs=6, space="PSUM") as maccp:

            h2aA = mp.tile([P, M1T, W2N], BF16, tag="h2aA")
            h2aB = mp.tile([P, M1T, W2N], BF16, tag="h2aB")

            def fetch_xq(pr):
                xq = mw.tile([P, KC, 2, N], F32, tag="xq")
                nc.sync.dma_start(
                    xq[:], xpd[2 * pr:2 * pr + 2]
                    .rearrange("u p s n -> p s u n"))
                xqb = mw.tile([P, KC, 2, N], BF16, tag="xqb")
                for s in range(KC):
                    nc.vector.tensor_copy(
                        xqb[:, s, :, :].rearrange("p u n -> p (u n)"),
                        xq[:, s, :, :].rearrange("p u n -> p (u n)"))
                return xq, xqb

            def ln2(xqb):
                xqbf = xqb.rearrange("p s u n -> p s (u n)")
                ms0 = mstp.tile([P, 512], F32, tag="st")
                ms1 = mstp.tile([P, 512], F32, tag="st")
                for s in range(KC):
                    xsq = mw.tile([P, W2N], BF16, tag="mxsq")
                    nc.vector.tensor_mul(xsq[:], xqbf[:, s, :], xqbf[:, s, :])
                    nc.tensor.matmul(ms0[:1, :], ones_c[:], xqbf[:, s, :],
                                     start=(s == 0), stop=(s == KC - 1))
                    nc.tensor.matmul(ms1[:1, :], ones_c[:], xsq[:],
                                     start=(s == 0), stop=(s == KC - 1))
                mean = mrows.tile([1, W2N], F32, tag="m_mean")
                ra = mrows.tile([1, W2N], F32, tag="m_a")
                rb = mrows.tile([1, W2N], F32, tag="m_b")
                nc.vector.tensor_scalar_mul(mean[:], ms0[:1, :], 1.0 / C)
                nc.vector.tensor_scalar_mul(ra[:], ms1[:1, :], 1.0 / C)
                nc.vector.tensor_mul(rb[:], mean[:], mean[:])
                nc.vector.tensor_sub(ra[:], ra[:], rb[:])
                nc.scalar.activation(rb[:], ra[:], AF.Sqrt, bias=EPS)
                nc.vector.reciprocal_approx_fast(ra[:], rb[:])
                rstd = mrows.tile([1, W2N], BF16, tag="m_rstd")
                nc.gpsimd.tensor_copy(rstd[:], ra[:])
                bneg = mrows.tile([1, W2N], BF16, tag="m_bneg")
                nc.vector.scalar_tensor_tensor(
                    bneg[:], mean[:], -1.0, rstd[:],
                    mybir.AluOpType.mult, mybir.AluOpType.mult)
                bc = mstp.tile([P, 512], F32, tag="st")
                nc.tensor.matmul(bc[:], ones_b[:1, :P], rstd[:],
                                 start=True, stop=True)
                bb = mstp.tile([P, 512], F32, tag="st")
                nc.tensor.matmul(bb[:], ones_b[:1, :P], bneg[:],
                                 start=True, stop=True)
                hp = mw.tile([P, KC, W2N], BF16, tag="hp")
                for s in range(KC):
                    nc.vector.tensor_mul(hp[:, s, :], xqbf[:, s, :], bc[:])
                    nc.vector.tensor_add(hp[:, s, :], hp[:, s, :], bb[:])
                return hp

            for sp in range(NPAIR // 2):
                prA, prB = 2 * sp, 2 * sp + 1
                xqA, xqbA = fetch_xq(prA)
                hpA = ln2(xqbA)
                xqB, xqbB = fetch_xq(prB)
                hpB = ln2(xqbB)
                # fc1 -> gelu for both pairs per weight tile
                for m1 in range(M1T):
                    w1t = mw1.tile([P, KC, P], BF16, tag="w1t")
                    nc.scalar.dma_start(w1t[:], w1[m1])
                    for hh, h2 in ((hpA, h2aA), (hpB, h2aB)):
                        ps1 = maccp.tile([P, 512], F32, tag="acc")
                        for k in range(KC):
                            nc.tensor.matmul(ps1[:], w1t[:, k, :],
                                             hh[:, k, :], start=(k == 0),
                                             stop=(k == KC - 1))
                        nc.scalar.activation(h2[:, m1, :], ps1[:],
                                             AF.Gelu_apprx_tanh,
                                             bias=f1bs[:, m1:m1 + 1])
                # fc2 + residual + output for both pairs per weight tile
                for pm in range(KC):
                    w2t = mw.tile([P, M1T, P], BF16, tag="w2t")
                    nc.gpsimd.dma_start(w2t[:, :M1T // 2, :],
                                        w2[pm, :, :M1T // 2, :])
                    nc.scalar.dma_start(w2t[:, M1T // 2:, :],
                                        w2[pm, :, M1T // 2:, :])
                    for pr, xqh, h2 in ((prA, xqA, h2aA), (prB, xqB, h2aB)):
                        ps2 = maccp.tile([P, 512], F32, tag="acc")
                        for m1 in range(M1T):
                            nc.tensor.matmul(ps2[:], w2t[:, m1, :],
                                             h2[:, m1, :],
                                             start=(m1 == 0),
                                             stop=(m1 == M1T - 1))
                        tb = mw3.tile([P, W2N], F32, tag="tb")
                        nc.scalar.activation(tb[:], ps2[:], AF.Identity,
                                             bias=b2s[:, KC + pm:KC + pm + 1])
                        ot = mw3.tile([P, 2, N], F32, tag="ot")
                        nc.vector.tensor_add(
                            ot[:], xqh[:, pm, :, :],
                            tb[:].rearrange("p (u n) -> p u n", n=N))
                        for u in range(2):
                            nc.gpsimd.dma_start(outT[2 * pr + u, :, pm, :],
                                                ot[:, u, :])

    nc.compile()
    return nc


# ---------------------------------------------------------------------------
# host side
# ---------------------------------------------------------------------------

def _qk_colmap():
    m = np.full(2 * H * HS, -1, np.int64)
    for h in range(H):
        m[HS * h:HS * h + DH] = np.arange(DH * h, DH * h + DH)
        m[H * HS + HS * h:H * HS + HS * h + DH] = \
            np.arange(C + DH * h, C + DH * h + DH)
    return m


def _prep_core_inputs(x_c, c_c, wdict):
    """x_c: [nw, N, C], c_c: [nw, C] -> per-core input map"""
    nw = x_c.shape[0]
    xT = np.ascontiguousarray(
        x_c.transpose(0, 2, 1).reshape(nw, KC, P, N).transpose(
            0, 2, 1, 3)).astype(np.float32)
    caug = np.zeros((nw, 1280), np.float32)
    caug[:, :C] = c_c
    caug[:, C] = 1.0
    cT = np.ascontiguousarray(caug.T.reshape(10, P, nw)).astype(NPBF16)
    return {"xT": xT, "xTb": xT.astype(NPBF16), "cT": cT, **wdict}


def _prep_weights(qkv_w, qkv_b, qkvt_w, qkvt_b, rpb_table, rel_idx,
                  proj_w, proj_b, fc1_w, fc1_b, fc2_w, fc2_b):
    qkmap = _qk_colmap()
    valid = qkmap >= 0

    # QK conditioning, chunked: wctc[ch, p, k, j] = wct[k*P+p, ch*256+j]
    wct = np.zeros((1280, 3072), np.float32)
    wct[:C, valid] = qkvt_w[qkmap[valid], :].T
    wct[C, valid] = (qkv_b + qkvt_b)[qkmap[valid]]
    wctc = np.ascontiguousarray(
        wct.reshape(10, P, 12, 2 * P).transpose(2, 1, 0, 3)).astype(NPBF16)

    # V conditioning, dense channels: wcvc[ch, p, k, j] = wcv[k*P+p, ch*256+j]
    wcv = np.zeros((1280, 1280), np.float32)
    wcv[:C, :C] = qkvt_w[2 * C:, :].T
    wcv[C, :C] = (qkv_b + qkvt_b)[2 * C:]
    wcvc = np.ascontiguousarray(
        wcv.reshape(10, P, 5, 2 * P).transpose(2, 1, 0, 3)).astype(NPBF16)

    nqk = 2 * H * HS
    wqkT = np.zeros((C, nqk), np.float32)
    wqkT[:, valid[:nqk]] = qkv_w[qkmap[valid[:nqk]], :].T
    # flat [P, QKM*KC*P]: tile (m, k)[p, j] = wqkT[k*P+p, m*P+j]
    wqk = np.ascontiguousarray(
        wqkT.reshape(KC, P, QKM, P).transpose(1, 2, 0, 3).reshape(
            P, QKM * KC * P)).astype(NPBF16)

    # flat [P, 4*KC*288]: tile (nch, k)[p, j] = wvT[k*P+p, nch*288+j]
    wvT = qkv_w[2 * C:, :].T
    wv = np.ascontiguousarray(
        wvT.reshape(KC, P, 4, 288).transpose(1, 2, 0, 3).reshape(
            P, 4 * KC * 288)).astype(NPBF16)

    bias = rpb_table[rel_idx]                      # [N(n), N(m), H]
    expb = np.ascontiguousarray(
        np.exp(bias).transpose(2, 1, 0).reshape(H, 2, P, N).transpose(
            0, 2, 1, 3)).astype(NPBF16)

    # dense proj flat [P, KC*KC*P]: tile (pc, k)[p, j] = proj_w[pc*P+j, k*P+p]
    wpd = np.zeros((P, KC * KC * P), np.float32)
    pw = proj_w.reshape(KC, P, KC, P)              # [pc, j, k, p]
    for pc in range(KC):
        for k in range(KC):
            wpd[:, (pc * KC + k) * P:(pc * KC + k + 1) * P] = pw[pc, :, k, :].T
    wpd = wpd.astype(NPBF16)

    permf = np.zeros((P, 24 * P), np.float32)
    pindf = np.zeros((1, 24 * P), np.float32)
    for idx, (pc, h, col0, r0, ln) in enumerate(_repack_pieces()):
        for d in range(ln):
            permf[r0 + d, idx * P + col0 + d] = 1.0
        pindf[0, idx * P + col0:idx * P + col0 + ln] = 1.0
    perm = permf.astype(NPBF16)
    pind = pindf.astype(NPBF16)

    if FP8_FC1:
        w1s = fc1_w * SW1                          # [MLP, C]
        w1p = np.zeros((M1T, KP1 * 2, P, P), np.float32)  # [m, kk, p, j]
        for m in range(M1T):
            for kk in range(KC):
                w1p[m, kk] = w1s[m * P:(m + 1) * P, kk * P:(kk + 1) * P].T
        w1 = np.ascontiguousarray(
            w1p.transpose(2, 0, 1, 3).reshape(P, M1T * KP1 * 2 * P)) \
            .astype(NPFP8)
    else:
        # [m, p, k, j] = fc1_w[m*P+j, k*P+p]
        w1 = np.ascontiguousarray(
            fc1_w.reshape(M1T, P, KC, P).transpose(0, 3, 2, 1)) \
            .astype(NPBF16)

    if FP8_FC2:
        w2s = fc2_w * SW2                          # [C, MLP]
        w2p = np.zeros((KC, KP2 * 2, P, P), np.float32)
        for pm in range(KC):
            for kk in range(M1T):
                w2p[pm, kk] = w2s[pm * P:(pm + 1) * P, kk * P:(kk + 1) * P].T
        w2 = np.ascontiguousarray(
            w2p.transpose(2, 0, 1, 3).reshape(P, KC * KP2 * 2 * P)) \
            .astype(NPFP8)
    else:
        w2 = np.ascontiguousarray(
            fc2_w.T.reshape(M1T, P, KC, P).transpose(2, 1, 0, 3)) \
            .astype(NPBF16)

    f1bv = np.ascontiguousarray(fc1_b.reshape(M1T, P).T).astype(np.float32)
    b2T = np.ascontiguousarray(
        np.concatenate([proj_b.reshape(KC, P), fc2_b.reshape(KC, P)])
        .T).astype(np.float32)

    return {"wctc": wctc, "wcvc": wcvc, "wqk": wqk, "wv": wv, "expb": expb,
            "wpd": wpd, "perm": perm, "pind": pind, "w1": w1, "w2": w2,
            "f1b": f1bv, "b2T": b2T}


_PROGRAM = None


def kernel(x, c, qkv_w, qkv_b, qkvt_w, qkvt_b, rpb_table, proj_w, proj_b,
           fc1_w, fc1_b, fc2_w, fc2_b, rel_idx, _trace=False):
    global _PROGRAM
    x = np.asarray(x, np.float32)
    c = np.asarray(c, np.float32)
    wdict = _prep_weights(
        np.asarray(qkv_w, np.float32), np.asarray(qkv_b, np.float32),
        np.asarray(qkvt_w, np.float32), np.asarray(qkvt_b, np.float32),
        np.asarray(rpb_table, np.float32), np.asarray(rel_idx),
        np.asarray(proj_w, np.float32), np.asarray(proj_b, np.float32),
        np.asarray(fc1_w, np.float32), np.asarray(fc1_b, np.float32),
        np.asarray(fc2_w, np.float32), np.asarray(fc2_b, np.float32))

    if _PROGRAM is None:
        _PROGRAM = build_program(NW)
    nc = _PROGRAM

    in_maps = []
    for core in range(NCORES):
        sl = slice(core * NW, (core + 1) * NW)
        in_maps.append(_prep_core_inputs(x[sl], c[sl], wdict))

    res = bass_utils.run_bass_kernel_spmd(
        nc, in_maps, core_ids=list(range(NCORES)), trace=_trace)

    out = np.empty((B, N, C), np.float32)
    for core in range(NCORES):
        oT = res.results[core]["outT"]            # [NW, P, KC, N]
        out[core * NW:(core + 1) * NW] = \
            oT.transpose(0, 2, 1, 3).reshape(NW, C, N).transpose(0, 2, 1)
    if _trace:
        return out, res
    return out
